# revision 1
# baseline (speedup 1.0000x reference)
"""GQA attention kernel for Trainium2 (Bass/Tile), 8-core SPMD.

Problem: B=2, N=2048, DIM=1024, 16 query heads / 4 KV heads, head_dim=64, fp32.
Sharding: core c = (batch b=c//4, kv-group g=c%4). Each core computes its
group's 4 query heads + 1 shared KV head over the full sequence, and a partial
output projection (its 256 rows of Wo). Host sums the 4 group partials per
batch and adds the bias.

Layout per core:
  xT    [128, 8, N] f32r : x^T (PE transposes with an f32r identity)
  qt    [128, 2, N] f32r : Q^T head pairs (head 2p on partitions 0-63, 2p+1 on
                           64-127)
  kkT   [128, N]    f32r : K^T duplicated across partition halves (DMA dup)
  vn    [128, 16, 65] bf16: V in normal layout (keys on partitions) + ones col
  aoutT [128, 2, N] f32r : normalized attention out^T for the out-projection

Scores are computed transposed (S^T [128 keys, 512 queries]); exp on Act; P@V
uses P^T tiles as the *stationary* operand and V as the moving operand,
producing [queries, 64] in PSUM at 64 rows/matmul instead of 128; sum-of-exp
rides on 1-row ones-matmuls into a dedicated PSUM bank.

PSUM budget (8 banks): scores 2x[128,1024] double-buffered (4) + P@V
accumulators 2x[128,512] (2, two heads per bank) + transpose staging (1) +
sum-of-exp (1). Projection matmuls share the score pool, interleaved
fine-grained between score tiles so no engine convoys behind one pool.
"""

import sys

if "/opt/trn_rl_repo" not in sys.path:
    sys.path.insert(0, "/opt/trn_rl_repo")

from collections import deque
from contextlib import ExitStack

import ml_dtypes
import numpy as np

BF16_NP = ml_dtypes.bfloat16

import concourse.bass as bass
import concourse.mybir as mybir
import concourse.tile as tile
from concourse import bacc, bass_utils
from concourse.bass import ds, ts
from concourse.masks import make_identity

F32 = mybir.dt.float32
F32R = mybir.dt.float32r
BF16 = mybir.dt.bfloat16
FP8 = mybir.dt.float8e4
DROW = mybir.MatmulPerfMode.DoubleRow
EXPF = mybir.ActivationFunctionType.Exp

DIM = 1024
D = 64  # head dim
SCALE = D ** -0.5


def build_nc(NSEQ=2048):
    KT = NSEQ // 128   # key tiles
    QC = NSEQ // 512   # query chunks of 512
    DKT = DIM // 128   # contraction tiles for projections

    nc = bacc.Bacc("TRN2", target_bir_lowering=False, debug=False)
    x = nc.dram_tensor("x", [NSEQ, DIM], F32, kind="ExternalInput").ap()
    wq = nc.dram_tensor("wq", [DIM, 256], BF16, kind="ExternalInput").ap()
    wkv = nc.dram_tensor("wkv", [DIM, 128], BF16, kind="ExternalInput").ap()
    wo = nc.dram_tensor("wo", [256, DIM], F32, kind="ExternalInput").ap()
    out = nc.dram_tensor("out", [DIM, NSEQ], F32, kind="ExternalOutput").ap()

    with tile.TileContext(nc) as tc, ExitStack() as ctx:
        sb = ctx.enter_context(tc.tile_pool(name="sb", bufs=1))

        wq_sb = sb.tile([128, DKT, 256], BF16)
        wkv_sb = sb.tile([128, DKT, 128], BF16)
        wo_sb = sb.tile([128, 2, DIM], F32R)
        ident = sb.tile([128, 128], F32)
        identr = sb.tile([128, 128], F32R)
        identb = sb.tile([128, 128], BF16)
        warm_in = sb.tile([128, 1], F32)
        warm = sb.tile([128, 1], F32)

        make_identity(nc, ident)
        nc.vector.tensor_copy(identr, ident)
        nc.vector.tensor_copy(identb, ident)
        nc.vector.memset(warm_in, 1.0)
        # preload the exp table set off the critical path
        nc.scalar.activation(out=warm, in_=warm_in, func=EXPF, scale=1.0)

        xT = sb.tile([128, DKT, NSEQ], BF16)
        qbT = sb.tile([128, 2, NSEQ], BF16)
        kbT = sb.tile([128, NSEQ], BF16)
        vn = sb.tile([128, KT, D + 1], BF16)
        aoutT = sb.tile([128, 2, NSEQ], F32R)
        nc.vector.memset(vn, 1.0)

        xpool = ctx.enter_context(tc.tile_pool(name="xp", bufs=3))
        xbp = ctx.enter_context(tc.tile_pool(name="xb", bufs=3))
        vtp = ctx.enter_context(tc.tile_pool(name="vtp", bufs=2))
        ptp = ctx.enter_context(tc.tile_pool(name="ptp", bufs=12))
        rrp = ctx.enter_context(tc.tile_pool(name="rrp", bufs=2))
        aop = ctx.enter_context(tc.tile_pool(name="aop", bufs=2))
        outp = ctx.enter_context(tc.tile_pool(name="outp", bufs=4))
        # PSUM: ps_sc 2x[128,1024] (banks 0-3), ps_pv 2x[128,512] (4-5),
        # ps_pj 1x[128,512] (6), ps_su 1x[128,16] (7)
        ps_sc = ctx.enter_context(tc.tile_pool(name="ps_sc", bufs=2, space="PSUM"))
        ps_pv = ctx.enter_context(tc.tile_pool(name="ps_pv", bufs=2, space="PSUM"))
        ps_pj = ctx.enter_context(tc.tile_pool(name="ps_pj", bufs=1, space="PSUM"))
        ps_su = ctx.enter_context(tc.tile_pool(name="ps_su", bufs=1, space="PSUM"))

        state = {}     # qc -> [hp0_tile, hp1_tile] each [128, 2, 4, 64] view
        sums = {}      # qc -> [128, 16] psum tile (cols h*4+i)
        pending_pv = []
        fillq = deque()

        def fill(n=1):
            for _ in range(n):
                if not fillq:
                    return
                fillq.popleft()()

        # ---------------- work units ----------------
        def unit_ptrq(sg):
            """Transpose x DIM-tiles 0-3 of chunk sg into one 2-bank bf16 psum."""
            def run():
                ptr = ps_sc.tile([128, 2048], BF16, tag="sc", name=f"ptrq{sg}")
                for d in range(4):
                    for i in range(4):
                        nc.tensor.transpose(ptr[:, ds(d * 512 + i * 128, 128)],
                                            xs_tiles[sg][:, i, ts(d, 128)], identb)
                nc.vector.tensor_copy(xT[:, ds(0, 4), ds(sg * 512, 512)], ptr)
            return run

        def unit_ptrp(sg, dp):
            """Transpose x DIM-tiles 2dp..2dp+1 into one 1-bank bf16 psum."""
            def run():
                ptr = ps_pj.tile([128, 1024], BF16, tag="pj", name=f"ptrp{sg}_{dp}")
                for k in range(2):
                    d = 2 * dp + k
                    for i in range(4):
                        nc.tensor.transpose(ptr[:, ds(k * 512 + i * 128, 128)],
                                            xs_tiles[sg][:, i, ts(d, 128)], identb)
                nc.vector.tensor_copy(xT[:, ds(2 * dp, 2), ds(sg * 512, 512)], ptr)
            return run

        def unit_pkv(sg):
            def run():
                pkv = ps_pj.tile([128, 512], F32, tag="pj", name=f"pkv{sg}")
                for d in range(DKT):
                    nc.tensor.matmul(pkv[:, 0:512], wkv_sb[:, d, :],
                                     xT[:, d, ds(sg * 512, 512)],
                                     start=(d == 0), stop=(d == DKT - 1))
                nc.vector.tensor_copy(kbT[ds(0, 64), ds(sg * 512, 512)],
                                      pkv[ds(0, 64), 0:512])
                nc.sync.dma_start(out=kbT[ds(64, 64), ds(sg * 512, 512)],
                                  in_=kbT[ds(0, 64), ds(sg * 512, 512)])
                vtmp = vtp.tile([64, 512], BF16, tag="vt", name=f"vt{sg}")
                nc.vector.tensor_copy(vtmp, pkv[ds(64, 64), 0:512])
                vtmp_tiles[sg] = vtmp
            return run

        def unit_ptv(sg):
            def run():
                ptv = ps_pj.tile([128, 1024], BF16, tag="pj", name=f"ptv{sg}")
                for i in range(4):
                    nc.tensor.transpose(ptv[:, ds(i * D, D)], vtmp_tiles[sg][:, ts(i, 128)],
                                        identb[0:64, 0:64])
                nc.vector.tensor_copy(vn[:, ds(sg * 4, 4), 0:D], ptv[:, 0:4 * D])
            return run

        def unit_qt(qc, p, hlf=None, pool=None):
            def run():
                if hlf is None:
                    qw, off = 512, 0
                else:
                    qw, off = 256, hlf * 256
                p_ = pool if pool is not None else ps_sc
                tag = "pj" if p_ is ps_pj else "sc"
                shape = [128, 512] if p_ is ps_pj else [128, 1024]
                pq = p_.tile(shape, F32, tag=tag, name=f"pq{qc}_{p}_{off}")
                for d in range(DKT):
                    nc.tensor.matmul(pq[:, 0:qw], wq_sb[:, d, ts(p, 128)],
                                     xT[:, d, ds(qc * 512 + off, qw)],
                                     start=(d == 0), stop=(d == DKT - 1))
                nc.vector.tensor_copy(qbT[:, p, ds(qc * 512 + off, qw)], pq[:, 0:qw])
            return run

        def unit_po(qc, od, pool=None, use_act=False):
            def run():
                p_ = pool if pool is not None else ps_sc
                tag = "pj" if p_ is ps_pj else "sc"
                shape = [128, 512] if p_ is ps_pj else [128, 1024]
                po = p_.tile(shape, F32, tag=tag, name=f"po{qc}_{od}")
                nc.tensor.matmul(po[:, 0:512], wo_sb[:, 0, ts(od, 128)],
                                 aoutT[:, 0, ds(qc * 512, 512)], start=True, stop=False)
                nc.tensor.matmul(po[:, 0:512], wo_sb[:, 1, ts(od, 128)],
                                 aoutT[:, 1, ds(qc * 512, 512)], start=False, stop=True)
                ot = outp.tile([128, 512], F32, tag="ot", name=f"ot{qc}_{od}")
                if use_act and od % 2 == 1:
                    nc.scalar.activation(out=ot, in_=po[:, 0:512],
                                         func=mybir.ActivationFunctionType.Copy,
                                         scale=1.0)
                else:
                    nc.vector.tensor_copy(ot, po[:, 0:512])
                nc.sync.dma_start(out=out[ts(od, 128), ds(qc * 512, 512)], in_=ot)
            return run

        # ---------------- attention ----------------
        def flush_pv():
            for (qc_, j_, h_, pt_) in pending_pv:
                hp, hh = h_ // 2, h_ % 2
                for t in range(2):
                    kt = 2 * j_ + t
                    for i in range(4):
                        stn = pt_[:, ds(t * 512 + i * 128, 128)]
                        # start=True zeroes the whole 2KB PSUM bank: only the
                        # first series touching each bank may set it.
                        nc.tensor.matmul(state[qc_][hp][:, hh, i, :], stn,
                                         vn[:, kt, 0:D],
                                         start=(kt == 0 and i == 0 and hh == 0),
                                         stop=(kt == KT - 1),
                                         skip_group_check=True)
                        nc.tensor.matmul(sums[qc_][:, ds(h_ * 4 + i, 1)], stn,
                                         vn[:, kt, D:D + 1],
                                         start=(kt == 0 and i == 0 and h_ == 0),
                                         stop=(kt == KT - 1),
                                         skip_group_check=True)
            pending_pv.clear()

        def emit_quanta(qc, j, mid_fills=(2,)):
            new_pv = []
            for h in range(4):
                p, i = h // 2, h % 2
                psc = ps_sc.tile([128, 1024], F32, tag="sc", name=f"psc{qc}_{j}_{h}")
                for t in range(2):
                    kt = 2 * j + t
                    nc.tensor.matmul(psc[:, ds(t * 512, 512)],
                                     kbT[ds(i * 64, 64), ts(kt, 128)],
                                     qbT[ds(i * 64, 64), p, ds(qc * 512, 512)],
                                     start=True, stop=True)
                pt = ptp.tile([128, 1024], BF16, tag="pt", name=f"pt{qc}_{j}_{h}")
                nc.scalar.activation(out=pt, in_=psc, func=EXPF, scale=SCALE)
                new_pv.append((qc, j, h, pt))
                if h == 1:
                    flush_pv()
                if h in mid_fills:
                    fill(1)
            pending_pv.extend(new_pv)

        def alloc_state(qc):
            state[qc] = [
                ps_pv.tile([128, 2, 4, D], F32, tag="pv", name=f"pv{qc}_{hp}")
                for hp in range(2)
            ]
            sums[qc] = ps_su.tile([128, 16], F32, tag="su", name=f"su{qc}")

        def emit_norm(qc, use_act=False):
            COPYF = mybir.ActivationFunctionType.Copy
            rr = rrp.tile([128, 16], F32, tag="rr", name=f"rr{qc}")
            nc.vector.reciprocal(out=rr, in_=sums[qc])
            ao = aop.tile([128, 4, 4, D], F32R, tag="ao", name=f"ao{qc}")
            for hp in range(2):
                for hh in range(2):
                    h = 2 * hp + hh
                    for i in range(4):
                        if use_act and (i % 2 == 1):
                            nc.scalar.activation(out=ao[:, i, h, :],
                                                 in_=state[qc][hp][:, hh, i, :],
                                                 func=COPYF,
                                                 scale=rr[:, ds(h * 4 + i, 1)])
                        else:
                            nc.vector.tensor_scalar_mul(ao[:, i, h, :],
                                                        state[qc][hp][:, hh, i, :],
                                                        rr[:, ds(h * 4 + i, 1)])
                pat = ps_pj.tile([128, 512], F32R, tag="pj", name=f"pat{qc}_{hp}")
                for i in range(4):
                    nc.tensor.transpose(pat[:, ds(i * 128, 128)],
                                        ao[:, i, ds(2 * hp, 2), :], identr)
                if use_act and hp == 1:
                    nc.scalar.activation(out=aoutT[:, hp, ds(qc * 512, 512)],
                                         in_=pat, func=COPYF, scale=1.0)
                else:
                    nc.vector.tensor_copy(aoutT[:, hp, ds(qc * 512, 512)], pat)

        # ---------------- schedule ----------------
        xs_tiles = {}
        vtmp_tiles = {}

        def dma_x(sg):
            if sg in xs_tiles:
                return
            xs = xpool.tile([128, 4, DIM], F32, tag="xs", name=f"xs{sg}")
            for hlf in range(2):
                src_ap = x[ds(sg * 512 + hlf * 256, 256), :].rearrange(
                    "(i p) m -> p i m", p=128)
                nc.sync.dma_start(out=xs[:, ds(hlf * 2, 2), :], in_=src_ap)
            xb = xbp.tile([128, 4, DIM], BF16, tag="xb", name=f"xb{sg}")
            COPYF = mybir.ActivationFunctionType.Copy
            for i in range(4):
                for hlf in range(2):
                    dst = xb[:, i, ds(hlf * 512, 512)]
                    srca = xs[:, i, ds(hlf * 512, 512)]
                    if sg < 2 and hlf == 0:
                        nc.scalar.activation(out=dst, in_=srca, func=COPYF, scale=1.0)
                    elif sg >= 2 and hlf == 1:
                        nc.vector.tensor_copy(dst, srca)
                    else:
                        nc.gpsimd.tensor_copy(dst, srca)
            xs_tiles[sg] = xb

        def sgroup_units(sg, eager):
            dma_x(sg)
            units = [unit_ptrq(sg), unit_ptrp(sg, 2), unit_ptrp(sg, 3),
                     unit_pkv(sg), unit_ptv(sg)]
            if eager:
                for u in units:
                    u()
            else:
                fillq.extend(units)

        # prologue: x chunk 0 first on the DMA engines, then weights
        dma_x(0)
        nc.sync.dma_start(out=wkv_sb, in_=wkv.rearrange("(t p) m -> p t m", p=128))
        nc.sync.dma_start(out=wq_sb, in_=wq.rearrange("(t p) m -> p t m", p=128))
        sgroup_units(0, eager=True)
        unit_qt(0, 0, 0)()
        unit_qt(0, 0, 1)()
        unit_qt(0, 1, 0)()
        unit_qt(0, 1, 1)()
        nc.sync.dma_start(out=wo_sb, in_=wo.rearrange("(t p) m -> p t m", p=128).bitcast(F32R))
        alloc_state(0)
        sgroup_units(1, eager=False)
        for j in range(KT // 2):
            if j == 0:
                dma_x(2)
            if j == 1:
                sgroup_units(2, eager=False)
            if j == 2:
                dma_x(3)
            if j == 3:
                sgroup_units(3, eager=False)
            if j >= 4 and j <= 7:
                fillq.append(unit_qt(1, (j - 4) // 2, (j - 4) % 2))
            emit_quanta(0, j, mid_fills=(1, 3))
            fill(1)
        for qc in range(1, QC):
            emit_quanta(qc, 0)       # flushes (qc-1, 7) into state[qc-1]
            emit_norm(qc - 1)
            alloc_state(qc)
            for j in range(1, KT // 2):
                fillq.append(unit_po(qc - 1, j - 1, ps_pj))
                if j >= 4 and qc + 1 < QC:
                    fillq.append(unit_qt(qc + 1, (j - 4) // 2, (j - 4) % 2, ps_pj))
                if j == 7:
                    fillq.append(unit_po(qc - 1, 7, ps_pj))
                emit_quanta(qc, j)
                fill(1)
        flush_pv()
        emit_norm(QC - 1, use_act=True)
        fill(len(fillq))
        for od in range(8):
            pool = ps_pj if od % 3 == 2 else ps_sc
            unit_po(QC - 1, od, pool, use_act=True)()

    nc.compile()
    return nc


_CACHE = {}


def _get_nc(NSEQ):
    if NSEQ not in _CACHE:
        _CACHE[NSEQ] = build_nc(NSEQ)
    return _CACHE[NSEQ]


def kernel(x, Wq, Wk, Wv, Wo, bo):
    """Full-input entry point: shard over 8 cores, run, gather."""
    x, Wq, Wk, Wv, Wo, bo = (np.asarray(a, np.float32) for a in (x, Wq, Wk, Wv, Wo, bo))
    B, N, C = x.shape
    nc = _get_nc(N)
    in_maps = []
    for c in range(8):
        b, g = c // 4, c % 4
        in_maps.append({
            "x": np.ascontiguousarray(x[b]),
            "wq": np.ascontiguousarray(Wq[:, g * 256:(g + 1) * 256]).astype(BF16_NP),
            "wkv": np.ascontiguousarray(np.concatenate(
                [Wk[:, g * D:(g + 1) * D], Wv[:, g * D:(g + 1) * D]],
                axis=1)).astype(BF16_NP),
            "wo": np.ascontiguousarray(Wo[g * 256:(g + 1) * 256, :]),
        })
    res = bass_utils.run_bass_kernel_spmd(nc, in_maps, core_ids=list(range(8)))
    outs = [res.results[c]["out"] for c in range(8)]
    full = np.empty((B, N, C), np.float32)
    for b in range(B):
        acc = outs[4 * b].astype(np.float32)
        for g in range(1, 4):
            acc = acc + outs[4 * b + g]
        full[b] = acc.T + bo[None, :]
    return full



# revision 37
# speedup vs baseline: 1.0121x; 1.0121x over previous
"""GQA attention kernel for Trainium2 (Bass/Tile), 8-core SPMD.

Problem: B=2, N=2048, DIM=1024, 16 query heads / 4 KV heads, head_dim=64, fp32.
Sharding: core c = (batch b=c//4, kv-group g=c%4). Each core computes its
group's 4 query heads + 1 shared KV head over the full sequence, and a partial
output projection (its 256 rows of Wo). Host sums the 4 group partials per
batch and adds the bias.

Engine plan (per-core busy targets, cost model). GPSIMD cannot touch PSUM on
TRN2, and every elementwise op here reads PSUM, so all of it splits between
Act and DVE:
  PE ~119us: scores (S^T, 128-key tiles x 512-query moving), P@V with P^T
    stationary, Q/KV/O projections, small V and aout transposes.
  Act ~100us: 80 of 128 exp tiles ([128,1024] PSUM->SBUF bf16) + the 16
    O-projection staging copies.
  DVE ~86us: 48 exp tiles via a single fused tensor_scalar op - Schraudolph
    bf16 exp: int16(out) = round(s*A + B) bit-cast to bf16 (max rel err ~4%
    on ~37% of keys -> ~1% on the final output) - plus projection-result
    copies, softmax normalization, reciprocal.
  DMA ~55us: x is loaded pre-transposed via dma_start_transpose (xbar,
    14ns/16x128 tile) straight into xT bf16 - no on-chip transpose pass.

PSUM (8 banks): scores 2x[128,1024]f32 (4) + P@V accumulators 2x[128,2,4,64]
(2) + projection staging [128,512] (1) + sum-of-exp [128,16] (1). Sum-of-exp
rides the PV matmuls via a ones column in vn. O-projection goes through the
score pool as [128,1024] od-pairs; Q-projection fills through the staging
bank at most once per quantum so bank-reuse handoffs stay off PE's critical
path.
"""

import sys

if "/opt/trn_rl_repo" not in sys.path:
    sys.path.insert(0, "/opt/trn_rl_repo")

from collections import deque
from contextlib import ExitStack

import ml_dtypes
import numpy as np

BF16_NP = ml_dtypes.bfloat16

import concourse.bass as bass
import concourse.mybir as mybir
import concourse.tile as tile
from concourse import bacc, bass_utils
from concourse.bass import ds, ts
from concourse.masks import make_identity

F32 = mybir.dt.float32
F32R = mybir.dt.float32r
BF16 = mybir.dt.bfloat16
I16 = mybir.dt.int16
EXPF = mybir.ActivationFunctionType.Exp
COPYF = mybir.ActivationFunctionType.Copy
MULT = mybir.AluOpType.mult
ADD = mybir.AluOpType.add

DIM = 1024
D = 64  # head dim
SCALE = D ** -0.5

# Schraudolph bf16 exp constants: bf16_bits(exp(s*SCALE)) ~= round(s*A + B)
A_SCH = SCALE * 128.0 / np.log(2.0)
B_SCH = 16256.0 - 6.5  # C=6.5 calibrated for round-to-nearest f32->int16


USE_SCHRAUDOLPH = True
USE_STT_NORM = True
DEBUG_DUMP = False
ACT_DUP = True
ACT_QBT = True


def build_nc(NSEQ=2048):
    KT = NSEQ // 128   # key tiles
    QC = NSEQ // 512   # query chunks of 512
    DKT = DIM // 128   # contraction tiles for projections

    nc = bacc.Bacc("TRN2", target_bir_lowering=False, debug=False)
    x = nc.dram_tensor("x", [NSEQ, DIM], BF16, kind="ExternalInput").ap()
    wqkv = nc.dram_tensor("wqkv", [DIM, 384], BF16, kind="ExternalInput").ap()
    wo = nc.dram_tensor("wo", [256, DIM], F32, kind="ExternalInput").ap()
    # Partials are stored bf16 (halves store DMA; host sums in f32 - adds
    # ~0.1% rms to one of four partials, well inside the error budget).
    out = nc.dram_tensor("out", [DIM, NSEQ], BF16, kind="ExternalOutput").ap()
    if DEBUG_DUMP:
        dbg_qbT = nc.dram_tensor("dbg_qbT", [128, 2, NSEQ], BF16,
                                 kind="ExternalOutput").ap()
        dbg_kbT = nc.dram_tensor("dbg_kbT", [128, NSEQ], BF16,
                                 kind="ExternalOutput").ap()
        dbg_vn = nc.dram_tensor("dbg_vn", [128, NSEQ // 128, 65], BF16,
                                kind="ExternalOutput").ap()
        dbg_aoutT = nc.dram_tensor("dbg_aoutT", [128, 2, NSEQ], mybir.dt.float32,
                                   kind="ExternalOutput").ap()
        dbg_xT = nc.dram_tensor("dbg_xT", [128, DIM // 128, NSEQ], BF16,
                                kind="ExternalOutput").ap()
        dbg_xT2 = nc.dram_tensor("dbg_xT2", [128, DIM // 128, NSEQ], BF16,
                                 kind="ExternalOutput").ap()

    with tile.TileContext(nc) as tc, ExitStack() as ctx:
        sb = ctx.enter_context(tc.tile_pool(name="sb", bufs=1))

        wqkv_sb = sb.tile([128, DKT, 384], BF16)
        wq_sb = wqkv_sb[:, :, 0:256]
        wkv_sb = wqkv_sb[:, :, 256:384]
        # wo loads as plain f32 (an F32R-bitcast DRAM AP corrupts any
        # in-flight xbar-transpose descriptors), then one DVE copy rounds it
        # into the F32R tile the O-projection matmuls consume.
        wo_f = sb.tile([128, 2, DIM], F32)
        wo_sb = sb.tile([128, 2, DIM], F32R)
        ident = sb.tile([128, 128], F32)
        identb = sb.tile([128, 128], BF16)
        identr = sb.tile([128, 128], F32R)
        warm_in = sb.tile([128, 1], F32)
        warm = sb.tile([128, 1], F32)

        xT = sb.tile([128, DKT, NSEQ], BF16)
        qbT = sb.tile([128, 2, NSEQ], BF16)
        kbT = sb.tile([128, NSEQ], BF16)
        vn = sb.tile([128, KT, D + 1], BF16)
        aoutT = sb.tile([128, 2, NSEQ], F32R)

        vtp = ctx.enter_context(tc.tile_pool(name="vtp", bufs=2))
        ptp = ctx.enter_context(tc.tile_pool(name="ptp", bufs=12))
        rrp = ctx.enter_context(tc.tile_pool(name="rrp", bufs=2))
        aop = ctx.enter_context(tc.tile_pool(name="aop", bufs=2))
        outp = ctx.enter_context(tc.tile_pool(name="outp", bufs=3))
        ps_sc = ctx.enter_context(tc.tile_pool(name="ps_sc", bufs=2, space="PSUM"))
        ps_pv = ctx.enter_context(tc.tile_pool(name="ps_pv", bufs=2, space="PSUM"))
        ps_pj = ctx.enter_context(tc.tile_pool(name="ps_pj", bufs=1, space="PSUM"))
        ps_su = ctx.enter_context(tc.tile_pool(name="ps_su", bufs=1, space="PSUM"))

        state = {}     # qc -> [hp0_tile, hp1_tile] each [128, 2, 4, 64]
        sums = {}      # qc -> [128, 16] psum tile (cols h*4+i)
        pending_pv = []
        fillq = deque()

        def fill(n=1):
            for _ in range(n):
                if not fillq:
                    return
                fillq.popleft()()

        # ---------------- work units ----------------
        def unit_pkv(sg, dup_eng):
            def run():
                pkv = ps_pj.tile([128, 512], F32, tag="pj", name=f"pkv{sg}")
                for d in range(DKT):
                    nc.tensor.matmul(pkv[:, 0:512], wkv_sb[:, d, :],
                                     xT[:, d, ds(sg * 512, 512)],
                                     start=(d == 0), stop=(d == DKT - 1))
                nc.vector.tensor_copy(kbT[ds(0, 64), ds(sg * 512, 512)],
                                      pkv[ds(0, 64), 0:512])
                eng = nc.scalar if (dup_eng == "act" and ACT_DUP) else nc.sync
                eng.dma_start(out=kbT[ds(64, 64), ds(sg * 512, 512)],
                              in_=kbT[ds(0, 64), ds(sg * 512, 512)])
                vtmp = vtp.tile([64, 512], BF16, tag="vt", name=f"vt{sg}")
                nc.vector.tensor_copy(vtmp, pkv[ds(64, 64), 0:512])
                vtmp_tiles[sg] = vtmp
            return run

        def unit_ptv(sg):
            def run():
                ptv = ps_pj.tile([128, 1024], BF16, tag="pj", name=f"ptv{sg}")
                for i in range(4):
                    nc.tensor.transpose(ptv[:, ds(i * D, D)],
                                        vtmp_tiles[sg][:, ts(i, 128)],
                                        identb[0:64, 0:64])
                nc.vector.tensor_copy(vn[:, ds(sg * 4, 4), 0:D], ptv[:, 0:4 * D])
            return run

        def unit_qt(qc, p, hlf=None, pool=None):
            def run():
                if hlf is None:
                    qw, off = 512, 0
                else:
                    qw, off = 256, hlf * 256
                p_ = pool if pool is not None else ps_pj
                tag = "pj" if p_ is ps_pj else "sc"
                shape = [128, 512] if p_ is ps_pj else [128, 1024]
                pq = p_.tile(shape, F32, tag=tag, name=f"pq{qc}_{p}_{off}")
                for d in range(DKT):
                    nc.tensor.matmul(pq[:, 0:qw], wq_sb[:, d, ts(p, 128)],
                                     xT[:, d, ds(qc * 512 + off, qw)],
                                     start=(d == 0), stop=(d == DKT - 1))
                if ACT_QBT:
                    nc.scalar.activation(out=qbT[:, p, ds(qc * 512 + off, qw)],
                                         in_=pq[:, 0:qw], func=COPYF, scale=1.0)
                else:
                    nc.vector.tensor_copy(qbT[:, p, ds(qc * 512 + off, qw)],
                                          pq[:, 0:qw])
            return run

        def unit_po_pair(qc, op, eng="act"):
            """O-projection for od = 2*op, 2*op+1 through a [128,1024] sc tile."""
            def run():
                po = ps_sc.tile([128, 1024], F32, tag="sc", name=f"po{qc}_{op}")
                for k in range(2):
                    od = 2 * op + k
                    nc.tensor.matmul(po[:, ds(k * 512, 512)],
                                     wo_sb[:, 0, ts(od, 128)],
                                     aoutT[:, 0, ds(qc * 512, 512)],
                                     start=True, stop=False)
                    nc.tensor.matmul(po[:, ds(k * 512, 512)],
                                     wo_sb[:, 1, ts(od, 128)],
                                     aoutT[:, 1, ds(qc * 512, 512)],
                                     start=False, stop=True)
                ot = outp.tile([128, 2, 512], BF16, tag="ot", name=f"ot{qc}_{op}")
                if eng == "act":
                    nc.scalar.activation(out=ot, in_=po, func=COPYF, scale=1.0)
                else:
                    nc.vector.tensor_copy(ot, po)
                nc.sync.dma_start(
                    out=out[ds(op * 256, 256), ds(qc * 512, 512)].rearrange(
                        "(t p) m -> p t m", p=128),
                    in_=ot)
            return run

        # ---------------- attention ----------------
        def flush_pv_one(qc_, j_, h_, pt_):
            hp, hh = h_ // 2, h_ % 2
            for t in range(2):
                kt = 2 * j_ + t
                for i in range(4):
                    stn = pt_[:, ds(t * 512 + i * 128, 128)]
                    # start=True zeroes the whole 2KB PSUM bank: only the
                    # first series touching each bank may set it.
                    nc.tensor.matmul(state[qc_][hp][:, hh, i, :], stn,
                                     vn[:, kt, 0:D],
                                     start=(kt == 0 and i == 0 and hh == 0),
                                     stop=(kt == KT - 1),
                                     skip_group_check=True)
                    nc.tensor.matmul(sums[qc_][:, ds(h_ * 4 + i, 1)], stn,
                                     vn[:, kt, D:D + 1],
                                     start=(kt == 0 and i == 0 and h_ == 0),
                                     stop=(kt == KT - 1),
                                     skip_group_check=True)

        def flush_pv():
            for (qc_, j_, h_, pt_) in pending_pv:
                flush_pv_one(qc_, j_, h_, pt_)
            pending_pv.clear()

        def exp_engine(qc, j, h):
            # Strict engine alternation: psc buffer k is reused two tiles
            # later, so exp(k) gates psc(k+2). With h0/h2 on DVE and h1/h3 on
            # Act, same-engine exps are two buffer-spacings apart and never
            # queue behind each other.
            return "dve" if h in (0, 2) else "act"

        def emit_quanta(qc, j, mid_fills=(3,)):
            # The previous quantum's P@V is interleaved per-head between this
            # quantum's score matmuls so PE has work inside every exp-wait.
            prev = list(pending_pv)
            pending_pv.clear()
            for h in range(4):
                p, i = h // 2, h % 2
                psc = ps_sc.tile([128, 1024], F32, tag="sc", name=f"psc{qc}_{j}_{h}")
                for t in range(2):
                    kt = 2 * j + t
                    nc.tensor.matmul(psc[:, ds(t * 512, 512)],
                                     kbT[ds(i * 64, 64), ts(kt, 128)],
                                     qbT[ds(i * 64, 64), p, ds(qc * 512, 512)],
                                     start=True, stop=True)
                pt = ptp.tile([128, 1024], BF16, tag="pt", name=f"pt{qc}_{j}_{h}")
                if not USE_SCHRAUDOLPH or exp_engine(qc, j, h) == "act":
                    nc.scalar.activation(out=pt, in_=psc, func=EXPF, scale=SCALE)
                else:
                    nc.vector.tensor_scalar(
                        out=pt.bitcast(I16), in0=psc, scalar1=A_SCH,
                        scalar2=B_SCH, op0=MULT, op1=ADD)
                pending_pv.append((qc, j, h, pt))
                if h < len(prev):
                    flush_pv_one(*prev[h])
                if h in mid_fills:
                    fill(1)

        def alloc_state(qc):
            state[qc] = [
                ps_pv.tile([128, 2, 4, D], F32, tag="pv", name=f"pv{qc}_{hp}")
                for hp in range(2)
            ]
            sums[qc] = ps_su.tile([128, 16], F32, tag="su", name=f"su{qc}")

        ao_tiles = {}

        def emit_norm_muls(qc):
            """Phase A: reciprocal + per-head scaling. One fused DVE op per
            head-pair: out = state * rr broadcast along the head-dim axis."""
            rr = rrp.tile([128, 16], F32, tag="rr", name=f"rr{qc}")
            nc.vector.reciprocal(out=rr, in_=sums[qc])
            # One fused op per head: out = state * rr broadcast along i,d.
            # ao layout [128, i, h, d] keeps each pat-transpose input slice
            # contiguous; per-head output [128, 4, 64] stays 3D for BIR.
            ao = aop.tile([128, 4, 4, D], F32R, tag="ao", name=f"ao{qc}")
            ao_tiles[qc] = ao
            for hp in range(2):
                for hh in range(2):
                    h = 2 * hp + hh
                    if USE_STT_NORM:
                        rr_b = rr[:, ds(4 * h, 4)].unsqueeze(-1).broadcast_to(
                            [128, 4, D])
                        nc.vector.scalar_tensor_tensor(
                            out=ao[:, :, h, :], in0=state[qc][hp][:, hh, :, :],
                            scalar=1.0, in1=rr_b, op0=MULT, op1=MULT)
                    else:
                        for i in range(4):
                            nc.vector.tensor_scalar_mul(
                                ao[:, i, h, :], state[qc][hp][:, hh, i, :],
                                rr[:, ds(h * 4 + i, 1)])

        def unit_patT(qc, hps=(0, 1), use_act=False):
            """Phase B: PE transposes of the normalized heads + aoutT copies.
            Dispatched a quantum after phase A so PE never chases the muls."""
            def run():
                ao = ao_tiles[qc]
                for hp in hps:
                    pat = ps_pj.tile([128, 512], F32R, tag="pj",
                                     name=f"pat{qc}_{hp}")
                    for i in range(4):
                        nc.tensor.transpose(pat[:, ds(i * 128, 128)],
                                            ao[:, i, ds(2 * hp, 2), :], identr)
                    if use_act and hp == 1:
                        nc.scalar.activation(
                            out=aoutT[:, hp, ds(qc * 512, 512)],
                            in_=pat, func=COPYF, scale=1.0)
                    else:
                        nc.vector.tensor_copy(
                            aoutT[:, hp, ds(qc * 512, 512)], pat)
            return run

        # ---------------- schedule ----------------
        vtmp_tiles = {}

        # DMA queue order sets the data-arrival schedule. Weight loads are
        # interleaved between the first x-transpose blocks so nothing waits
        # a full 8-deep DMA batch; x chunks 2-3 are issued as fill units
        # during qc0 so later DMAs (kbT dup, stores) don't queue behind them.
        def dma_xt(sg, ts_=range(8)):
            for t in ts_:
                nc.sync.dma_start_transpose(
                    out=xT[:, t, ds(sg * 512, 512)],
                    in_=x[ds(sg * 512, 512), ds(t * 128, 128)])

        def unit_xt(sg):
            def run():
                dma_xt(sg)
            return run

        nc.sync.dma_start(out=wqkv_sb,
                          in_=wqkv.rearrange("(t p) m -> p t m", p=128))
        dma_xt(0)
        dma_xt(1)
        dma_xt(2)
        nc.sync.dma_start(out=wo_f,
                          in_=wo.rearrange("(t p) m -> p t m", p=128))
        dma_xt(3)

        make_identity(nc, ident)
        nc.vector.tensor_copy(identb, ident)
        nc.vector.tensor_copy(identr, ident)
        nc.vector.tensor_copy(wo_sb, wo_f)
        nc.vector.memset(vn, 1.0)
        nc.vector.memset(warm_in, 1.0)
        # preload the exp table set off the critical path
        nc.scalar.activation(out=warm, in_=warm_in, func=EXPF, scale=1.0)

        if DEBUG_DUMP:
            nc.sync.dma_start(out=dbg_xT2, in_=xT)
        unit_pkv(0, "act")()
        unit_qt(0, 0, pool=ps_sc)()
        unit_qt(0, 1, pool=ps_sc)()
        unit_ptv(0)()
        unit_pkv(1, "act")()
        unit_ptv(1)()
        alloc_state(0)

        # qc 0: kbT chunk s is consumed from j=2s; chunks 2-3 land mid-loop.
        fills_at = {2: [unit_pkv(2, "sync")], 3: [unit_ptv(2)],
                    4: [unit_pkv(3, "sync")],
                    5: [unit_ptv(3), unit_qt(1, 0, 0)],
                    6: [unit_qt(1, 0, 1), unit_qt(1, 1, 0)],
                    7: [unit_qt(1, 1, 1)]}
        for j in range(KT // 2):
            fillq.extend(fills_at.get(j, []))
            emit_quanta(0, j)
            fill(1)
        for qc in range(1, QC):
            emit_quanta(qc, 0)       # flushes (qc-1, 7) into state[qc-1]
            emit_norm_muls(qc - 1)
            alloc_state(qc)
            fills_qc = {1: [unit_patT(qc - 1)],
                        2: [unit_po_pair(qc - 1, 0, "act")],
                        3: [unit_po_pair(qc - 1, 1, "act")],
                        4: [unit_po_pair(qc - 1, 2, "act")],
                        5: [unit_po_pair(qc - 1, 3, "act")]}
            if qc + 1 < QC:
                fills_qc[6] = [unit_qt(qc + 1, 0, 0), unit_qt(qc + 1, 0, 1)]
                fills_qc[7] = [unit_qt(qc + 1, 1, 0), unit_qt(qc + 1, 1, 1)]
            for j in range(1, KT // 2):
                fillq.extend(fills_qc.get(j, []))
                emit_quanta(qc, j)
                fill(1)
        flush_pv()
        emit_norm_muls(QC - 1)
        unit_patT(QC - 1, use_act=True)()  # both head pairs
        fill(len(fillq))
        for op in range(4):
            unit_po_pair(QC - 1, op, "act" if op % 2 == 0 else "dve")()
        if DEBUG_DUMP:
            nc.sync.dma_start(out=dbg_qbT, in_=qbT)
            nc.sync.dma_start(out=dbg_kbT, in_=kbT)
            nc.sync.dma_start(out=dbg_vn, in_=vn)
            nc.sync.dma_start(out=dbg_aoutT, in_=aoutT.bitcast(F32))
            nc.sync.dma_start(out=dbg_xT, in_=xT)

    nc.compile()
    return nc


_CACHE = {}


def _get_nc(NSEQ):
    if NSEQ not in _CACHE:
        _CACHE[NSEQ] = build_nc(NSEQ)
    return _CACHE[NSEQ]


def kernel(x, Wq, Wk, Wv, Wo, bo):
    """Full-input entry point: shard over 8 cores, run, gather."""
    x, Wq, Wk, Wv, Wo, bo = (np.asarray(a, np.float32) for a in (x, Wq, Wk, Wv, Wo, bo))
    B, N, C = x.shape
    nc = _get_nc(N)
    in_maps = []
    for c in range(8):
        b, g = c // 4, c % 4
        in_maps.append({
            "x": np.ascontiguousarray(x[b]).astype(BF16_NP),
            "wqkv": np.ascontiguousarray(np.concatenate(
                [Wq[:, g * 256:(g + 1) * 256],
                 Wk[:, g * D:(g + 1) * D], Wv[:, g * D:(g + 1) * D]],
                axis=1)).astype(BF16_NP),
            "wo": np.ascontiguousarray(Wo[g * 256:(g + 1) * 256, :]),
        })
    res = bass_utils.run_bass_kernel_spmd(nc, in_maps, core_ids=list(range(8)))
    outs = [res.results[c]["out"] for c in range(8)]
    full = np.empty((B, N, C), np.float32)
    for b in range(B):
        acc = outs[4 * b].astype(np.float32)
        for g in range(1, 4):
            acc = acc + outs[4 * b + g]
        full[b] = acc.T + bo[None, :]
    return full


# revision 45
# speedup vs baseline: 1.0665x; 1.0537x over previous
"""GQA attention kernel for Trainium2 (Bass/Tile), 8-core SPMD.

Problem: B=2, N=2048, DIM=1024, 16 query heads / 4 KV heads, head_dim=64, fp32.
Sharding: core c = (batch b=c//4, kv-group g=c%4). Each core computes its
group's 4 query heads + 1 shared KV head over the full sequence, and a partial
output projection (its 256 rows of Wo). Host sums the 4 group partials per
batch and adds the bias.

Engine plan (per-core busy targets, cost model). GPSIMD cannot touch PSUM on
TRN2, and every elementwise op here reads PSUM, so all of it splits between
Act and DVE:
  PE ~119us: scores (S^T, 128-key tiles x 512-query moving), P@V with P^T
    stationary, Q/KV/O projections, small V and aout transposes.
  Act ~100us: 80 of 128 exp tiles ([128,1024] PSUM->SBUF bf16) + the 16
    O-projection staging copies.
  DVE ~86us: 48 exp tiles via a single fused tensor_scalar op - Schraudolph
    bf16 exp: int16(out) = round(s*A + B) bit-cast to bf16 (max rel err ~4%
    on ~37% of keys -> ~1% on the final output) - plus projection-result
    copies, softmax normalization, reciprocal.
  DMA ~55us: x is loaded pre-transposed via dma_start_transpose (xbar,
    14ns/16x128 tile) straight into xT bf16 - no on-chip transpose pass.

PSUM (8 banks): scores 2x[128,1024]f32 (4) + P@V accumulators 2x[128,2,4,64]
(2) + projection staging [128,512] (1) + sum-of-exp [128,16] (1). Sum-of-exp
rides the PV matmuls via a ones column in vn. O-projection goes through the
score pool as [128,1024] od-pairs; Q-projection fills through the staging
bank at most once per quantum so bank-reuse handoffs stay off PE's critical
path.
"""

import sys

if "/opt/trn_rl_repo" not in sys.path:
    sys.path.insert(0, "/opt/trn_rl_repo")

from collections import deque
from contextlib import ExitStack

import ml_dtypes
import numpy as np

BF16_NP = ml_dtypes.bfloat16

import concourse.bass as bass
import concourse.mybir as mybir
import concourse.tile as tile
from concourse import bacc, bass_utils
from concourse.bass import ds, ts
from concourse.masks import make_identity

F32 = mybir.dt.float32
F32R = mybir.dt.float32r
BF16 = mybir.dt.bfloat16
I16 = mybir.dt.int16
EXPF = mybir.ActivationFunctionType.Exp
COPYF = mybir.ActivationFunctionType.Copy
MULT = mybir.AluOpType.mult
ADD = mybir.AluOpType.add

DIM = 1024
D = 64  # head dim
SCALE = D ** -0.5

# Schraudolph bf16 exp constants: bf16_bits(exp(s*SCALE)) ~= round(s*A + B)
A_SCH = SCALE * 128.0 / np.log(2.0)
B_SCH = 16256.0 - 6.5  # C=6.5 calibrated for round-to-nearest f32->int16


USE_SCHRAUDOLPH = True
USE_STT_NORM = True
DEBUG_DUMP = False
ACT_DUP = True
ACT_QBT = True


def build_nc(NSEQ=2048):
    KT = NSEQ // 128   # key tiles
    QC = NSEQ // 512   # query chunks of 512
    DKT = DIM // 128   # contraction tiles for projections

    nc = bacc.Bacc("TRN2", target_bir_lowering=False, debug=False)
    x = nc.dram_tensor("x", [NSEQ, DIM], BF16, kind="ExternalInput").ap()
    wqkv = nc.dram_tensor("wqkv", [DIM, 384], BF16, kind="ExternalInput").ap()
    wo = nc.dram_tensor("wo", [256, DIM], BF16, kind="ExternalInput").ap()
    # Partials are stored bf16 (halves store DMA; host sums in f32 - adds
    # ~0.1% rms to one of four partials, well inside the error budget).
    out = nc.dram_tensor("out", [DIM, NSEQ], BF16, kind="ExternalOutput").ap()
    if DEBUG_DUMP:
        dbg_qbT = nc.dram_tensor("dbg_qbT", [128, 2, NSEQ], BF16,
                                 kind="ExternalOutput").ap()
        dbg_kbT = nc.dram_tensor("dbg_kbT", [128, NSEQ], BF16,
                                 kind="ExternalOutput").ap()
        dbg_vn = nc.dram_tensor("dbg_vn", [128, NSEQ // 128, 65], BF16,
                                kind="ExternalOutput").ap()
        dbg_aoutT = nc.dram_tensor("dbg_aoutT", [128, 2, NSEQ], mybir.dt.float32,
                                   kind="ExternalOutput").ap()
        dbg_xT = nc.dram_tensor("dbg_xT", [128, DIM // 128, NSEQ], BF16,
                                kind="ExternalOutput").ap()
        dbg_xT2 = nc.dram_tensor("dbg_xT2", [128, DIM // 128, NSEQ], BF16,
                                 kind="ExternalOutput").ap()

    with tile.TileContext(nc) as tc, ExitStack() as ctx:
        sb = ctx.enter_context(tc.tile_pool(name="sb", bufs=1))

        wqkv_sb = sb.tile([128, DKT, 384], BF16)
        wq_sb = wqkv_sb[:, :, 0:256]
        wkv_sb = wqkv_sb[:, :, 256:384]
        # wo is bf16: stationary dtype doesn't affect matmul cost, and an
        # F32R DRAM tensor (or bitcast AP) corrupts in-flight xbar-transpose
        # descriptors, so plain bf16 is both faster to load and safe.
        wo_sb = sb.tile([128, 2, DIM], BF16)
        ident = sb.tile([128, 128], F32)
        identb = sb.tile([128, 128], BF16)
        identr = sb.tile([128, 128], F32R)
        warm_in = sb.tile([128, 1], F32)
        warm = sb.tile([128, 1], F32)

        xT = sb.tile([128, DKT, NSEQ], BF16)
        qbT = sb.tile([128, 2, NSEQ], BF16)
        kbT = sb.tile([128, NSEQ], BF16)
        vn = sb.tile([128, KT, D + 1], BF16)
        aoutT = sb.tile([128, 2, NSEQ], BF16)

        vtp = ctx.enter_context(tc.tile_pool(name="vtp", bufs=2))
        ptp = ctx.enter_context(tc.tile_pool(name="ptp", bufs=12))
        rrp = ctx.enter_context(tc.tile_pool(name="rrp", bufs=2))
        aop = ctx.enter_context(tc.tile_pool(name="aop", bufs=2))
        outp = ctx.enter_context(tc.tile_pool(name="outp", bufs=3))
        ps_sc = ctx.enter_context(tc.tile_pool(name="ps_sc", bufs=2, space="PSUM"))
        ps_pv = ctx.enter_context(tc.tile_pool(name="ps_pv", bufs=2, space="PSUM"))
        ps_pj = ctx.enter_context(tc.tile_pool(name="ps_pj", bufs=1, space="PSUM"))
        ps_su = ctx.enter_context(tc.tile_pool(name="ps_su", bufs=1, space="PSUM"))

        state = {}     # qc -> [hp0_tile, hp1_tile] each [128, 2, 4, 64]
        sums = {}      # qc -> [128, 16] psum tile (cols h*4+i)
        pending_pv = []
        fillq = deque()

        def fill(n=1):
            for _ in range(n):
                if not fillq:
                    return
                fillq.popleft()()

        # ---------------- work units ----------------
        def unit_pkv(sg, dup_eng):
            def run():
                pkv = ps_pj.tile([128, 512], F32, tag="pj", name=f"pkv{sg}")
                for d in range(DKT):
                    nc.tensor.matmul(pkv[:, 0:512], wkv_sb[:, d, :],
                                     xT[:, d, ds(sg * 512, 512)],
                                     start=(d == 0), stop=(d == DKT - 1))
                nc.vector.tensor_copy(kbT[ds(0, 64), ds(sg * 512, 512)],
                                      pkv[ds(0, 64), 0:512])
                eng = nc.scalar if (dup_eng == "act" and ACT_DUP) else nc.sync
                eng.dma_start(out=kbT[ds(64, 64), ds(sg * 512, 512)],
                              in_=kbT[ds(0, 64), ds(sg * 512, 512)])
                vtmp = vtp.tile([64, 512], BF16, tag="vt", name=f"vt{sg}")
                nc.vector.tensor_copy(vtmp, pkv[ds(64, 64), 0:512])
                vtmp_tiles[sg] = vtmp
            return run

        def unit_ptv(sg):
            def run():
                ptv = ps_pj.tile([128, 1024], BF16, tag="pj", name=f"ptv{sg}")
                for i in range(4):
                    nc.tensor.transpose(ptv[:, ds(i * D, D)],
                                        vtmp_tiles[sg][:, ts(i, 128)],
                                        identb[0:64, 0:64])
                nc.vector.tensor_copy(vn[:, ds(sg * 4, 4), 0:D], ptv[:, 0:4 * D])
            return run

        def unit_qt(qc, p, hlf=None, pool=None):
            def run():
                if hlf is None:
                    qw, off = 512, 0
                else:
                    qw, off = 256, hlf * 256
                p_ = pool if pool is not None else ps_pj
                tag = "pj" if p_ is ps_pj else "sc"
                shape = [128, 512] if p_ is ps_pj else [128, 1024]
                pq = p_.tile(shape, F32, tag=tag, name=f"pq{qc}_{p}_{off}")
                for d in range(DKT):
                    nc.tensor.matmul(pq[:, 0:qw], wq_sb[:, d, ts(p, 128)],
                                     xT[:, d, ds(qc * 512 + off, qw)],
                                     start=(d == 0), stop=(d == DKT - 1))
                if ACT_QBT:
                    nc.scalar.activation(out=qbT[:, p, ds(qc * 512 + off, qw)],
                                         in_=pq[:, 0:qw], func=COPYF, scale=1.0)
                else:
                    nc.vector.tensor_copy(qbT[:, p, ds(qc * 512 + off, qw)],
                                          pq[:, 0:qw])
            return run

        def unit_po_pair(qc, op, eng="act"):
            """O-projection for od = 2*op, 2*op+1 through a [128,1024] sc tile."""
            def run():
                po = ps_sc.tile([128, 1024], F32, tag="sc", name=f"po{qc}_{op}")
                for k in range(2):
                    od = 2 * op + k
                    nc.tensor.matmul(po[:, ds(k * 512, 512)],
                                     wo_sb[:, 0, ts(od, 128)],
                                     aoutT[:, 0, ds(qc * 512, 512)],
                                     start=True, stop=False)
                    nc.tensor.matmul(po[:, ds(k * 512, 512)],
                                     wo_sb[:, 1, ts(od, 128)],
                                     aoutT[:, 1, ds(qc * 512, 512)],
                                     start=False, stop=True)
                ot = outp.tile([128, 2, 512], BF16, tag="ot", name=f"ot{qc}_{op}")
                if eng == "act":
                    nc.scalar.activation(out=ot, in_=po, func=COPYF, scale=1.0)
                else:
                    nc.vector.tensor_copy(ot, po)
                nc.sync.dma_start(
                    out=out[ds(op * 256, 256), ds(qc * 512, 512)].rearrange(
                        "(t p) m -> p t m", p=128),
                    in_=ot)
            return run

        # ---------------- attention ----------------
        def flush_pv_one(qc_, j_, h_, pt_):
            # Flush order within a quantum is h0, h1, h3, h2 (see
            # emit_quanta), so the first series to touch the hp1 state bank
            # is hh==1. start=True zeroes the whole 2KB PSUM bank: only that
            # first series may set it.
            hp, hh = h_ // 2, h_ % 2
            first_hh = 1 if hp == 1 else 0
            for t in range(2):
                kt = 2 * j_ + t
                for i in range(4):
                    stn = pt_[:, ds(t * 512 + i * 128, 128)]
                    nc.tensor.matmul(state[qc_][hp][:, hh, i, :], stn,
                                     vn[:, kt, 0:D],
                                     start=(kt == 0 and i == 0
                                            and hh == first_hh),
                                     stop=(kt == KT - 1),
                                     skip_group_check=True)
                    nc.tensor.matmul(sums[qc_][:, ds(h_ * 4 + i, 1)], stn,
                                     vn[:, kt, D:D + 1],
                                     start=(kt == 0 and i == 0 and h_ == 0),
                                     stop=(kt == KT - 1),
                                     skip_group_check=True)

        def flush_pv():
            for (qc_, j_, h_, pt_) in pending_pv:
                flush_pv_one(qc_, j_, h_, pt_)
            pending_pv.clear()

        def exp_engine(qc, j, h):
            # Strict engine alternation: psc buffer k is reused two tiles
            # later, so exp(k) gates psc(k+2). With h0/h2 on DVE and h1/h3 on
            # Act, same-engine exps are two buffer-spacings apart and never
            # queue behind each other.
            return "dve" if h in (0, 2) else "act"

        def emit_quanta(qc, j, mid_fills=(3,)):
            # The previous quantum's P@V is interleaved per-head between this
            # quantum's score matmuls so PE has work inside every exp-wait.
            prev = list(pending_pv)
            pending_pv.clear()
            for h in range(4):
                p, i = h // 2, h % 2
                psc = ps_sc.tile([128, 1024], F32, tag="sc", name=f"psc{qc}_{j}_{h}")
                for t in range(2):
                    kt = 2 * j + t
                    nc.tensor.matmul(psc[:, ds(t * 512, 512)],
                                     kbT[ds(i * 64, 64), ts(kt, 128)],
                                     qbT[ds(i * 64, 64), p, ds(qc * 512, 512)],
                                     start=True, stop=True)
                pt = ptp.tile([128, 1024], BF16, tag="pt", name=f"pt{qc}_{j}_{h}")
                if not USE_SCHRAUDOLPH or exp_engine(qc, j, h) == "act":
                    nc.scalar.activation(out=pt, in_=psc, func=EXPF, scale=SCALE)
                else:
                    nc.vector.tensor_scalar(
                        out=pt.bitcast(I16), in0=psc, scalar1=A_SCH,
                        scalar2=B_SCH, op0=MULT, op1=ADD)
                pending_pv.append((qc, j, h, pt))
                # flush order h0, h1, h3, h2: the DVE-run exps (h0/h2) gate
                # the two-buffer psc rotation, so the even-head score matmuls
                # get extra PE work in front of them.
                if prev:
                    if h < 2:
                        flush_pv_one(*prev[h])
                    elif h == 2:
                        flush_pv_one(*prev[2])
                if h == 1 and prev:
                    flush_pv_one(*prev[3])
                if h in mid_fills:
                    fill(1)

        def alloc_state(qc):
            state[qc] = [
                ps_pv.tile([128, 2, 4, D], F32, tag="pv", name=f"pv{qc}_{hp}")
                for hp in range(2)
            ]
            sums[qc] = ps_su.tile([128, 16], F32, tag="su", name=f"su{qc}")

        ao_tiles = {}

        def emit_norm_muls(qc):
            """Phase A: reciprocal + per-head scaling. One fused DVE op per
            head-pair: out = state * rr broadcast along the head-dim axis."""
            rr = rrp.tile([128, 16], F32, tag="rr", name=f"rr{qc}")
            nc.vector.reciprocal(out=rr, in_=sums[qc])
            # One fused op per head: out = state * rr broadcast along i,d.
            # ao layout [128, i, h, d] keeps each pat-transpose input slice
            # contiguous; per-head output [128, 4, 64] stays 3D for BIR.
            ao = aop.tile([128, 4, 4, D], F32R, tag="ao", name=f"ao{qc}")
            ao_tiles[qc] = ao
            for hp in range(2):
                for hh in range(2):
                    h = 2 * hp + hh
                    if USE_STT_NORM:
                        rr_b = rr[:, ds(4 * h, 4)].unsqueeze(-1).broadcast_to(
                            [128, 4, D])
                        nc.vector.scalar_tensor_tensor(
                            out=ao[:, :, h, :], in0=state[qc][hp][:, hh, :, :],
                            scalar=1.0, in1=rr_b, op0=MULT, op1=MULT)
                    else:
                        for i in range(4):
                            nc.vector.tensor_scalar_mul(
                                ao[:, i, h, :], state[qc][hp][:, hh, i, :],
                                rr[:, ds(h * 4 + i, 1)])

        def unit_patT(qc, hps=(0, 1), use_act=False):
            """Phase B: PE transposes of the normalized heads + aoutT copies.
            Dispatched a quantum after phase A so PE never chases the muls."""
            def run():
                ao = ao_tiles[qc]
                for hp in hps:
                    pat = ps_pj.tile([128, 512], F32R, tag="pj",
                                     name=f"pat{qc}_{hp}")
                    for i in range(4):
                        nc.tensor.transpose(pat[:, ds(i * 128, 128)],
                                            ao[:, i, ds(2 * hp, 2), :], identr)
                    if use_act and hp == 1:
                        nc.scalar.activation(
                            out=aoutT[:, hp, ds(qc * 512, 512)],
                            in_=pat, func=COPYF, scale=1.0)
                    else:
                        nc.vector.tensor_copy(
                            aoutT[:, hp, ds(qc * 512, 512)], pat)
            return run

        # ---------------- schedule ----------------
        vtmp_tiles = {}

        # DMA queue order sets the data-arrival schedule. Weight loads are
        # interleaved between the first x-transpose blocks so nothing waits
        # a full 8-deep DMA batch; x chunks 2-3 are issued as fill units
        # during qc0 so later DMAs (kbT dup, stores) don't queue behind them.
        def dma_xt(sp):
            # one transpose DMA per (chunk-pair, 128-col block): 16 DMAs
            # total instead of 32 - HWDGE issue overhead (~0.65us per DMA)
            # paces x delivery at startup.
            for t in range(DKT):
                nc.sync.dma_start_transpose(
                    out=xT[:, t, ds(sp * 1024, 1024)],
                    in_=x[ds(sp * 1024, 1024), ds(t * 128, 128)])

        nc.sync.dma_start(out=wqkv_sb,
                          in_=wqkv.rearrange("(t p) m -> p t m", p=128))
        dma_xt(0)
        dma_xt(1)
        nc.sync.dma_start(out=wo_sb,
                          in_=wo.rearrange("(t p) m -> p t m", p=128))

        make_identity(nc, ident)
        nc.vector.tensor_copy(identb, ident)
        nc.vector.tensor_copy(identr, ident)
        nc.vector.memset(vn, 1.0)
        nc.vector.memset(warm_in, 1.0)
        # preload the exp table set off the critical path
        nc.scalar.activation(out=warm, in_=warm_in, func=EXPF, scale=1.0)

        if DEBUG_DUMP:
            nc.sync.dma_start(out=dbg_xT2, in_=xT)
        unit_pkv(0, "act")()
        unit_qt(0, 0, pool=ps_sc)()
        unit_qt(0, 1, pool=ps_sc)()
        unit_ptv(0)()
        unit_pkv(1, "act")()
        unit_ptv(1)()
        alloc_state(0)

        # qc 0: kbT chunk s is consumed from j=2s; chunks 2-3 land mid-loop.
        fills_at = {2: [unit_pkv(2, "sync")], 3: [unit_ptv(2)],
                    4: [unit_pkv(3, "sync")],
                    5: [unit_ptv(3), unit_qt(1, 0, 0)],
                    6: [unit_qt(1, 0, 1), unit_qt(1, 1, 0)],
                    7: [unit_qt(1, 1, 1)]}
        for j in range(KT // 2):
            fillq.extend(fills_at.get(j, []))
            emit_quanta(0, j)
            fill(1)
        for qc in range(1, QC):
            emit_quanta(qc, 0)       # flushes (qc-1, 7) into state[qc-1]
            emit_norm_muls(qc - 1)
            alloc_state(qc)
            fills_qc = {1: [unit_patT(qc - 1)],
                        2: [unit_po_pair(qc - 1, 0, "act")],
                        3: [unit_po_pair(qc - 1, 1, "act")],
                        4: [unit_po_pair(qc - 1, 2, "act")],
                        5: [unit_po_pair(qc - 1, 3, "act")]}
            if qc + 1 < QC:
                fills_qc[6] = [unit_qt(qc + 1, 0, 0), unit_qt(qc + 1, 0, 1)]
                fills_qc[7] = [unit_qt(qc + 1, 1, 0), unit_qt(qc + 1, 1, 1)]
            for j in range(1, KT // 2):
                fillq.extend(fills_qc.get(j, []))
                emit_quanta(qc, j)
                fill(1)
        flush_pv()
        emit_norm_muls(QC - 1)
        unit_patT(QC - 1, use_act=True)()  # both head pairs
        fill(len(fillq))
        for op in range(4):
            unit_po_pair(QC - 1, op, "act" if op % 2 == 0 else "dve")()
        if DEBUG_DUMP:
            nc.sync.dma_start(out=dbg_qbT, in_=qbT)
            nc.sync.dma_start(out=dbg_kbT, in_=kbT)
            nc.sync.dma_start(out=dbg_vn, in_=vn)
            nc.sync.dma_start(out=dbg_aoutT, in_=aoutT.bitcast(F32))
            nc.sync.dma_start(out=dbg_xT, in_=xT)

    nc.compile()
    return nc


_CACHE = {}


def _get_nc(NSEQ):
    if NSEQ not in _CACHE:
        _CACHE[NSEQ] = build_nc(NSEQ)
    return _CACHE[NSEQ]


def kernel(x, Wq, Wk, Wv, Wo, bo):
    """Full-input entry point: shard over 8 cores, run, gather."""
    x, Wq, Wk, Wv, Wo, bo = (np.asarray(a, np.float32) for a in (x, Wq, Wk, Wv, Wo, bo))
    B, N, C = x.shape
    nc = _get_nc(N)
    in_maps = []
    for c in range(8):
        b, g = c // 4, c % 4
        in_maps.append({
            "x": np.ascontiguousarray(x[b]).astype(BF16_NP),
            "wqkv": np.ascontiguousarray(np.concatenate(
                [Wq[:, g * 256:(g + 1) * 256],
                 Wk[:, g * D:(g + 1) * D], Wv[:, g * D:(g + 1) * D]],
                axis=1)).astype(BF16_NP),
            "wo": np.ascontiguousarray(
                Wo[g * 256:(g + 1) * 256, :]).astype(BF16_NP),
        })
    res = bass_utils.run_bass_kernel_spmd(nc, in_maps, core_ids=list(range(8)))
    outs = [res.results[c]["out"] for c in range(8)]
    full = np.empty((B, N, C), np.float32)
    for b in range(B):
        acc = outs[4 * b].astype(np.float32)
        for g in range(1, 4):
            acc = acc + outs[4 * b + g]
        full[b] = acc.T + bo[None, :]
    return full


# revision 50
# speedup vs baseline: 1.0827x; 1.0152x over previous
"""GQA attention kernel for Trainium2 (Bass/Tile), 8-core SPMD.

Problem: B=2, N=2048, DIM=1024, 16 query heads / 4 KV heads, head_dim=64, fp32.
Sharding: core c = (batch b=c//4, kv-group g=c%4). Each core computes its
group's 4 query heads + 1 shared KV head over the full sequence, and a partial
output projection (its 256 rows of Wo). Host sums the 4 group partials per
batch and adds the bias.

Engine plan (per-core busy targets, cost model). GPSIMD cannot touch PSUM on
TRN2, and every elementwise op here reads PSUM, so all of it splits between
Act and DVE:
  PE ~119us: scores (S^T, 128-key tiles x 512-query moving), P@V with P^T
    stationary, Q/KV/O projections, small V and aout transposes.
  Act ~100us: 80 of 128 exp tiles ([128,1024] PSUM->SBUF bf16) + the 16
    O-projection staging copies.
  DVE ~86us: 48 exp tiles via a single fused tensor_scalar op - Schraudolph
    bf16 exp: int16(out) = round(s*A + B) bit-cast to bf16 (max rel err ~4%
    on ~37% of keys -> ~1% on the final output) - plus projection-result
    copies, softmax normalization, reciprocal.
  DMA ~55us: x is loaded pre-transposed via dma_start_transpose (xbar,
    14ns/16x128 tile) straight into xT bf16 - no on-chip transpose pass.

PSUM (8 banks): scores 2x[128,1024]f32 (4) + P@V accumulators 2x[128,2,4,64]
(2) + projection staging [128,512] (1) + sum-of-exp [128,16] (1). Sum-of-exp
rides the PV matmuls via a ones column in vn. O-projection goes through the
score pool as [128,1024] od-pairs; Q-projection fills through the staging
bank at most once per quantum so bank-reuse handoffs stay off PE's critical
path.
"""

import sys

if "/opt/trn_rl_repo" not in sys.path:
    sys.path.insert(0, "/opt/trn_rl_repo")

from collections import deque
from contextlib import ExitStack

import ml_dtypes
import numpy as np

BF16_NP = ml_dtypes.bfloat16

import concourse.bass as bass
import concourse.mybir as mybir
import concourse.tile as tile
from concourse import bacc, bass_utils
from concourse.bass import ds, ts
from concourse.masks import make_identity

F32 = mybir.dt.float32
F32R = mybir.dt.float32r
BF16 = mybir.dt.bfloat16
I16 = mybir.dt.int16
EXPF = mybir.ActivationFunctionType.Exp
COPYF = mybir.ActivationFunctionType.Copy
MULT = mybir.AluOpType.mult
ADD = mybir.AluOpType.add

DIM = 1024
D = 64  # head dim
SCALE = D ** -0.5

# Schraudolph bf16 exp constants: bf16_bits(exp(s*SCALE)) ~= round(s*A + B)
A_SCH = SCALE * 128.0 / np.log(2.0)
B_SCH = 16256.0 - 6.5  # C=6.5 calibrated for round-to-nearest f32->int16


USE_SCHRAUDOLPH = True
USE_STT_NORM = True
DEBUG_DUMP = False
ACT_DUP = True
ACT_QBT = True


def build_nc(NSEQ=2048):
    KT = NSEQ // 128   # key tiles
    QC = NSEQ // 512   # query chunks of 512
    DKT = DIM // 128   # contraction tiles for projections

    nc = bacc.Bacc("TRN2", target_bir_lowering=False, debug=False)
    x = nc.dram_tensor("x", [NSEQ, DIM], BF16, kind="ExternalInput").ap()
    wqkv = nc.dram_tensor("wqkv", [DIM, 384], BF16, kind="ExternalInput").ap()
    wo = nc.dram_tensor("wo", [256, DIM], BF16, kind="ExternalInput").ap()
    # Partials are stored bf16 (halves store DMA; host sums in f32 - adds
    # ~0.1% rms to one of four partials, well inside the error budget).
    out = nc.dram_tensor("out", [DIM, NSEQ], BF16, kind="ExternalOutput").ap()
    if DEBUG_DUMP:
        dbg_qbT = nc.dram_tensor("dbg_qbT", [128, 2, NSEQ], BF16,
                                 kind="ExternalOutput").ap()
        dbg_kbT = nc.dram_tensor("dbg_kbT", [128, NSEQ], BF16,
                                 kind="ExternalOutput").ap()
        dbg_vn = nc.dram_tensor("dbg_vn", [128, NSEQ // 128, 65], BF16,
                                kind="ExternalOutput").ap()
        dbg_aoutT = nc.dram_tensor("dbg_aoutT", [128, 2, NSEQ], mybir.dt.float32,
                                   kind="ExternalOutput").ap()
        dbg_xT = nc.dram_tensor("dbg_xT", [128, DIM // 128, NSEQ], BF16,
                                kind="ExternalOutput").ap()
        dbg_xT2 = nc.dram_tensor("dbg_xT2", [128, DIM // 128, NSEQ], BF16,
                                 kind="ExternalOutput").ap()

    with tile.TileContext(nc) as tc, ExitStack() as ctx:
        sb = ctx.enter_context(tc.tile_pool(name="sb", bufs=1))

        wqkv_sb = sb.tile([128, DKT, 384], BF16)
        wq_sb = wqkv_sb[:, :, 0:256]
        wkv_sb = wqkv_sb[:, :, 256:384]
        # wo is bf16: stationary dtype doesn't affect matmul cost, and an
        # F32R DRAM tensor (or bitcast AP) corrupts in-flight xbar-transpose
        # descriptors, so plain bf16 is both faster to load and safe.
        wo_sb = sb.tile([128, 2, DIM], BF16)
        ident = sb.tile([128, 128], F32)
        identb = sb.tile([128, 128], BF16)
        identr = sb.tile([128, 128], F32R)
        warm_in = sb.tile([128, 1], F32)
        warm = sb.tile([128, 1], F32)

        xT = sb.tile([128, DKT, NSEQ], BF16)
        qbT = sb.tile([128, 2, NSEQ], BF16)
        kbT = sb.tile([128, NSEQ], BF16)
        vn = sb.tile([128, KT, D + 1], BF16)
        aoutT = sb.tile([128, 2, NSEQ], BF16)

        vtp = ctx.enter_context(tc.tile_pool(name="vtp", bufs=2))
        ptp = ctx.enter_context(tc.tile_pool(name="ptp", bufs=12))
        rrp = ctx.enter_context(tc.tile_pool(name="rrp", bufs=2))
        aop = ctx.enter_context(tc.tile_pool(name="aop", bufs=2))
        outp = ctx.enter_context(tc.tile_pool(name="outp", bufs=4))
        ps_sc = ctx.enter_context(tc.tile_pool(name="ps_sc", bufs=2, space="PSUM"))
        ps_pv = ctx.enter_context(tc.tile_pool(name="ps_pv", bufs=2, space="PSUM"))
        ps_pj = ctx.enter_context(tc.tile_pool(name="ps_pj", bufs=1, space="PSUM"))
        ps_su = ctx.enter_context(tc.tile_pool(name="ps_su", bufs=1, space="PSUM"))

        state = {}     # qc -> [hp0_tile, hp1_tile] each [128, 2, 4, 64]
        sums = {}      # qc -> [128, 16] psum tile (cols h*4+i)
        rr_tiles = {}
        pending_pv = []
        fillq = deque()

        def fill(n=1):
            for _ in range(n):
                if not fillq:
                    return
                fillq.popleft()()

        # ---------------- work units ----------------
        def unit_pkv(sg, dup_eng):
            def run():
                pkv = ps_pj.tile([128, 512], F32, tag="pj", name=f"pkv{sg}")
                for d in range(DKT):
                    nc.tensor.matmul(pkv[:, 0:512], wkv_sb[:, d, :],
                                     xT[:, d, ds(sg * 512, 512)],
                                     start=(d == 0), stop=(d == DKT - 1))
                nc.vector.tensor_copy(kbT[ds(0, 64), ds(sg * 512, 512)],
                                      pkv[ds(0, 64), 0:512])
                eng = nc.scalar if (dup_eng == "act" and ACT_DUP) else nc.sync
                eng.dma_start(out=kbT[ds(64, 64), ds(sg * 512, 512)],
                              in_=kbT[ds(0, 64), ds(sg * 512, 512)])
                vtmp = vtp.tile([64, 512], BF16, tag="vt", name=f"vt{sg}")
                nc.vector.tensor_copy(vtmp, pkv[ds(64, 64), 0:512])
                vtmp_tiles[sg] = vtmp
            return run

        def unit_ptv(sg):
            def run():
                ptv = ps_pj.tile([128, 1024], BF16, tag="pj", name=f"ptv{sg}")
                for i in range(4):
                    nc.tensor.transpose(ptv[:, ds(i * D, D)],
                                        vtmp_tiles[sg][:, ts(i, 128)],
                                        identb[0:64, 0:64])
                nc.vector.tensor_copy(vn[:, ds(sg * 4, 4), 0:D], ptv[:, 0:4 * D])
            return run

        def unit_qt(qc, p, hlf=None, pool=None):
            def run():
                if hlf is None:
                    qw, off = 512, 0
                else:
                    qw, off = 256, hlf * 256
                p_ = pool if pool is not None else ps_pj
                tag = "pj" if p_ is ps_pj else "sc"
                shape = [128, 512] if p_ is ps_pj else [128, 1024]
                pq = p_.tile(shape, F32, tag=tag, name=f"pq{qc}_{p}_{off}")
                for d in range(DKT):
                    nc.tensor.matmul(pq[:, 0:qw], wq_sb[:, d, ts(p, 128)],
                                     xT[:, d, ds(qc * 512 + off, qw)],
                                     start=(d == 0), stop=(d == DKT - 1))
                if ACT_QBT:
                    nc.scalar.activation(out=qbT[:, p, ds(qc * 512 + off, qw)],
                                         in_=pq[:, 0:qw], func=COPYF, scale=1.0)
                else:
                    nc.vector.tensor_copy(qbT[:, p, ds(qc * 512 + off, qw)],
                                          pq[:, 0:qw])
            return run

        def unit_po_pair(qc, op, eng="act"):
            """O-projection for od = 2*op, 2*op+1 through a [128,1024] sc tile."""
            def run():
                po = ps_sc.tile([128, 1024], F32, tag="sc", name=f"po{qc}_{op}")
                for k in range(2):
                    od = 2 * op + k
                    nc.tensor.matmul(po[:, ds(k * 512, 512)],
                                     wo_sb[:, 0, ts(od, 128)],
                                     aoutT[:, 0, ds(qc * 512, 512)],
                                     start=True, stop=False)
                    nc.tensor.matmul(po[:, ds(k * 512, 512)],
                                     wo_sb[:, 1, ts(od, 128)],
                                     aoutT[:, 1, ds(qc * 512, 512)],
                                     start=False, stop=True)
                ot = outp.tile([128, 2, 512], BF16, tag="ot", name=f"ot{qc}_{op}")
                if eng == "act":
                    nc.scalar.activation(out=ot, in_=po, func=COPYF, scale=1.0)
                else:
                    nc.vector.tensor_copy(ot, po)
                nc.sync.dma_start(
                    out=out[ds(op * 256, 256), ds(qc * 512, 512)].rearrange(
                        "(t p) m -> p t m", p=128),
                    in_=ot)
            return run

        # ---------------- attention ----------------
        def flush_pv_one(qc_, j_, h_, pt_):
            # Flush order within a quantum is h0, h1, h3, h2 (see
            # emit_quanta), so the first series to touch the hp1 state bank
            # is hh==1. start=True zeroes the whole 2KB PSUM bank: only that
            # first series may set it.
            hp, hh = h_ // 2, h_ % 2
            first_hh = 1 if hp == 1 else 0
            for t in range(2):
                kt = 2 * j_ + t
                for i in range(4):
                    stn = pt_[:, ds(t * 512 + i * 128, 128)]
                    nc.tensor.matmul(state[qc_][hp][:, hh, i, :], stn,
                                     vn[:, kt, 0:D],
                                     start=(kt == 0 and i == 0
                                            and hh == first_hh),
                                     stop=(kt == KT - 1),
                                     skip_group_check=True)
                    nc.tensor.matmul(sums[qc_][:, ds(h_ * 4 + i, 1)], stn,
                                     vn[:, kt, D:D + 1],
                                     start=(kt == 0 and i == 0 and h_ == 0),
                                     stop=(kt == KT - 1),
                                     skip_group_check=True)

        def flush_pv():
            for (qc_, j_, h_, pt_) in pending_pv:
                flush_pv_one(qc_, j_, h_, pt_)
            pending_pv.clear()

        def exp_engine(qc, j, h):
            # Strict engine alternation: psc buffer k is reused two tiles
            # later, so exp(k) gates psc(k+2). With h0/h2 on DVE and h1/h3 on
            # Act, same-engine exps are two buffer-spacings apart and never
            # queue behind each other.
            return "dve" if h in (0, 2) else "act"

        def emit_quanta(qc, j, mid_fills=(3,)):
            # The previous quantum's P@V is interleaved per-head between this
            # quantum's score matmuls so PE has work inside every exp-wait.
            prev = list(pending_pv)
            pending_pv.clear()
            for h in range(4):
                p, i = h // 2, h % 2
                psc = ps_sc.tile([128, 1024], F32, tag="sc", name=f"psc{qc}_{j}_{h}")
                for t in range(2):
                    kt = 2 * j + t
                    nc.tensor.matmul(psc[:, ds(t * 512, 512)],
                                     kbT[ds(i * 64, 64), ts(kt, 128)],
                                     qbT[ds(i * 64, 64), p, ds(qc * 512, 512)],
                                     start=True, stop=True)
                pt = ptp.tile([128, 1024], BF16, tag="pt", name=f"pt{qc}_{j}_{h}")
                if not USE_SCHRAUDOLPH or exp_engine(qc, j, h) == "act":
                    nc.scalar.activation(out=pt, in_=psc, func=EXPF, scale=SCALE)
                else:
                    nc.vector.tensor_scalar(
                        out=pt.bitcast(I16), in0=psc, scalar1=A_SCH,
                        scalar2=B_SCH, op0=MULT, op1=ADD)
                pending_pv.append((qc, j, h, pt))
                # flush order h0, h1, h3, h2: the DVE-run exps (h0/h2) gate
                # the two-buffer psc rotation, so the even-head score matmuls
                # get extra PE work in front of them.
                if prev:
                    if h < 2:
                        flush_pv_one(*prev[h])
                    elif h == 2:
                        flush_pv_one(*prev[2])
                if h == 1 and prev:
                    flush_pv_one(*prev[3])
                if h in mid_fills:
                    fill(1)

        def alloc_state(qc):
            state[qc] = [
                ps_pv.tile([128, 2, 4, D], F32, tag="pv", name=f"pv{qc}_{hp}")
                for hp in range(2)
            ]
            sums[qc] = ps_su.tile([128, 16], F32, tag="su", name=f"su{qc}")

        ao_tiles = {}

        def norm_muls(qc, hps):
            """Per-head scaling: one fused DVE op per head, out = state * rr
            broadcast along i,d. ao layout [128, i, h, d] keeps each
            pat-transpose input slice contiguous; per-head output stays 3D."""
            ao = ao_tiles[qc]
            rr = rr_tiles[qc]
            for hp in hps:
                for hh in range(2):
                    h = 2 * hp + hh
                    rr_b = rr[:, ds(4 * h, 4)].unsqueeze(-1).broadcast_to(
                        [128, 4, D])
                    nc.vector.scalar_tensor_tensor(
                        out=ao[:, :, h, :], in0=state[qc][hp][:, hh, :, :],
                        scalar=1.0, in1=rr_b, op0=MULT, op1=MULT)

        def emit_norm_muls(qc):
            """Phase A: reciprocal + hp0 scaling; hp1's muls ride the j1
            fill to smooth DVE's qc-boundary load."""
            rr = rrp.tile([128, 16], F32, tag="rr", name=f"rr{qc}")
            nc.vector.reciprocal(out=rr, in_=sums[qc])
            rr_tiles[qc] = rr
            ao_tiles[qc] = aop.tile([128, 4, 4, D], F32R, tag="ao",
                                    name=f"ao{qc}")
            norm_muls(qc, (0, 1))

        def unit_patT(qc, hps=(0, 1), use_act=False, pre_muls=()):
            """Phase B: PE transposes of the normalized heads + aoutT copies.
            Dispatched a quantum after phase A so PE never chases the muls."""
            def run():
                norm_muls(qc, pre_muls)
                ao = ao_tiles[qc]
                for hp in hps:
                    pat = ps_pj.tile([128, 512], F32R, tag="pj",
                                     name=f"pat{qc}_{hp}")
                    for i in range(4):
                        nc.tensor.transpose(pat[:, ds(i * 128, 128)],
                                            ao[:, i, ds(2 * hp, 2), :], identr)
                    # Act takes both aoutT copies: DVE is the exp
                    # bottleneck at qc boundaries.
                    nc.scalar.activation(
                        out=aoutT[:, hp, ds(qc * 512, 512)],
                        in_=pat, func=COPYF, scale=1.0)
            return run

        # ---------------- schedule ----------------
        vtmp_tiles = {}

        # DMA queue order sets the data-arrival schedule. Weight loads are
        # interleaved between the first x-transpose blocks so nothing waits
        # a full 8-deep DMA batch; x chunks 2-3 are issued as fill units
        # during qc0 so later DMAs (kbT dup, stores) don't queue behind them.
        def dma_xt(sp):
            # one transpose DMA per (chunk-pair, 128-col block): 16 DMAs
            # total instead of 32 - HWDGE issue overhead (~0.65us per DMA)
            # paces x delivery at startup.
            for t in range(DKT):
                nc.sync.dma_start_transpose(
                    out=xT[:, t, ds(sp * 1024, 1024)],
                    in_=x[ds(sp * 1024, 1024), ds(t * 128, 128)])

        nc.sync.dma_start(out=wqkv_sb,
                          in_=wqkv.rearrange("(t p) m -> p t m", p=128))
        dma_xt(0)
        dma_xt(1)
        nc.sync.dma_start(out=wo_sb,
                          in_=wo.rearrange("(t p) m -> p t m", p=128))

        make_identity(nc, ident)
        nc.vector.tensor_copy(identb, ident)
        nc.vector.tensor_copy(identr, ident)
        nc.vector.memset(vn, 1.0)
        nc.vector.memset(warm_in, 1.0)
        # preload the exp table set off the critical path
        nc.scalar.activation(out=warm, in_=warm_in, func=EXPF, scale=1.0)

        if DEBUG_DUMP:
            nc.sync.dma_start(out=dbg_xT2, in_=xT)
        unit_pkv(0, "act")()
        unit_qt(0, 0, pool=ps_sc)()
        unit_qt(0, 1, pool=ps_sc)()
        unit_ptv(0)()
        unit_pkv(1, "act")()
        unit_ptv(1)()
        alloc_state(0)

        # qc 0: kbT chunk s is consumed from j=2s; chunks 2-3 land mid-loop.
        fills_at = {2: [unit_pkv(2, "sync")], 3: [unit_ptv(2)],
                    4: [unit_pkv(3, "sync")],
                    5: [unit_ptv(3), unit_qt(1, 0, 0)],
                    6: [unit_qt(1, 0, 1), unit_qt(1, 1, 0)],
                    7: [unit_qt(1, 1, 1)]}
        for j in range(KT // 2):
            fillq.extend(fills_at.get(j, []))
            emit_quanta(0, j)
            fill(1)
        for qc in range(1, QC):
            emit_quanta(qc, 0)       # flushes (qc-1, 7) into state[qc-1]
            emit_norm_muls(qc - 1)
            alloc_state(qc)
            fills_qc = {1: [unit_patT(qc - 1)],
                        2: [unit_po_pair(qc - 1, 0, "act")],
                        3: [unit_po_pair(qc - 1, 1, "act")],
                        4: [unit_po_pair(qc - 1, 2, "act")],
                        5: [unit_po_pair(qc - 1, 3, "act")]}
            if qc + 1 < QC:
                fills_qc[6] = [unit_qt(qc + 1, 0, 0), unit_qt(qc + 1, 0, 1)]
                fills_qc[7] = [unit_qt(qc + 1, 1, 0), unit_qt(qc + 1, 1, 1)]
            for j in range(1, KT // 2):
                fillq.extend(fills_qc.get(j, []))
                emit_quanta(qc, j)
                fill(1)
        flush_pv()
        emit_norm_muls(QC - 1)
        unit_patT(QC - 1, use_act=True)()  # both head pairs
        fill(len(fillq))
        for op in range(4):
            unit_po_pair(QC - 1, op, "act" if op % 2 == 0 else "dve")()
        if DEBUG_DUMP:
            nc.sync.dma_start(out=dbg_qbT, in_=qbT)
            nc.sync.dma_start(out=dbg_kbT, in_=kbT)
            nc.sync.dma_start(out=dbg_vn, in_=vn)
            nc.sync.dma_start(out=dbg_aoutT, in_=aoutT.bitcast(F32))
            nc.sync.dma_start(out=dbg_xT, in_=xT)

    nc.compile()
    return nc


_CACHE = {}


def _get_nc(NSEQ):
    if NSEQ not in _CACHE:
        _CACHE[NSEQ] = build_nc(NSEQ)
    return _CACHE[NSEQ]


def kernel(x, Wq, Wk, Wv, Wo, bo):
    """Full-input entry point: shard over 8 cores, run, gather."""
    x, Wq, Wk, Wv, Wo, bo = (np.asarray(a, np.float32) for a in (x, Wq, Wk, Wv, Wo, bo))
    B, N, C = x.shape
    nc = _get_nc(N)
    in_maps = []
    for c in range(8):
        b, g = c // 4, c % 4
        in_maps.append({
            "x": np.ascontiguousarray(x[b]).astype(BF16_NP),
            "wqkv": np.ascontiguousarray(np.concatenate(
                [Wq[:, g * 256:(g + 1) * 256],
                 Wk[:, g * D:(g + 1) * D], Wv[:, g * D:(g + 1) * D]],
                axis=1)).astype(BF16_NP),
            "wo": np.ascontiguousarray(
                Wo[g * 256:(g + 1) * 256, :]).astype(BF16_NP),
        })
    res = bass_utils.run_bass_kernel_spmd(nc, in_maps, core_ids=list(range(8)))
    outs = [res.results[c]["out"] for c in range(8)]
    full = np.empty((B, N, C), np.float32)
    for b in range(B):
        acc = outs[4 * b].astype(np.float32)
        for g in range(1, 4):
            acc = acc + outs[4 * b + g]
        full[b] = acc.T + bo[None, :]
    return full


# revision 55
# speedup vs baseline: 1.0828x; 1.0001x over previous
"""GQA attention kernel for Trainium2 (Bass/Tile), 8-core SPMD.

Problem: B=2, N=2048, DIM=1024, 16 query heads / 4 KV heads, head_dim=64, fp32.
Sharding: core c = (batch b=c//4, kv-group g=c%4). Each core computes its
group's 4 query heads + 1 shared KV head over the full sequence, and a partial
output projection (its 256 rows of Wo). Host sums the 4 group partials per
batch and adds the bias.

Engine plan (per-core busy targets, cost model). GPSIMD cannot touch PSUM on
TRN2, and every elementwise op here reads PSUM, so all of it splits between
Act and DVE:
  PE ~119us: scores (S^T, 128-key tiles x 512-query moving), P@V with P^T
    stationary, Q/KV/O projections, small V and aout transposes.
  Act ~100us: 80 of 128 exp tiles ([128,1024] PSUM->SBUF bf16) + the 16
    O-projection staging copies.
  DVE ~86us: 48 exp tiles via a single fused tensor_scalar op - Schraudolph
    bf16 exp: int16(out) = round(s*A + B) bit-cast to bf16 (max rel err ~4%
    on ~37% of keys -> ~1% on the final output) - plus projection-result
    copies, softmax normalization, reciprocal.
  DMA ~55us: x is loaded pre-transposed via dma_start_transpose (xbar,
    14ns/16x128 tile) straight into xT bf16 - no on-chip transpose pass.

PSUM (8 banks): scores 2x[128,1024]f32 (4) + P@V accumulators 2x[128,2,4,64]
(2) + projection staging [128,512] (1) + sum-of-exp [128,16] (1). Sum-of-exp
rides the PV matmuls via a ones column in vn. O-projection goes through the
score pool as [128,1024] od-pairs; Q-projection fills through the staging
bank at most once per quantum so bank-reuse handoffs stay off PE's critical
path.
"""

import sys

if "/opt/trn_rl_repo" not in sys.path:
    sys.path.insert(0, "/opt/trn_rl_repo")

from collections import deque
from contextlib import ExitStack

import ml_dtypes
import numpy as np

BF16_NP = ml_dtypes.bfloat16

import concourse.bass as bass
import concourse.mybir as mybir
import concourse.tile as tile
from concourse import bacc, bass_utils
from concourse.bass import ds, ts
from concourse.masks import make_identity

F32 = mybir.dt.float32
F32R = mybir.dt.float32r
BF16 = mybir.dt.bfloat16
I16 = mybir.dt.int16
EXPF = mybir.ActivationFunctionType.Exp
COPYF = mybir.ActivationFunctionType.Copy
MULT = mybir.AluOpType.mult
ADD = mybir.AluOpType.add

DIM = 1024
D = 64  # head dim
SCALE = D ** -0.5

# Schraudolph bf16 exp constants: bf16_bits(exp(s*SCALE)) ~= round(s*A + B)
A_SCH = SCALE * 128.0 / np.log(2.0)
B_SCH = 16256.0 - 6.5  # C=6.5 calibrated for round-to-nearest f32->int16


USE_SCHRAUDOLPH = True
USE_STT_NORM = True
DEBUG_DUMP = False
ACT_DUP = True
ACT_QBT = True


def build_nc(NSEQ=2048):
    KT = NSEQ // 128   # key tiles
    QC = NSEQ // 512   # query chunks of 512
    DKT = DIM // 128   # contraction tiles for projections

    nc = bacc.Bacc("TRN2", target_bir_lowering=False, debug=False)
    x = nc.dram_tensor("x", [NSEQ, DIM], BF16, kind="ExternalInput").ap()
    wqkv = nc.dram_tensor("wqkv", [DIM, 384], BF16, kind="ExternalInput").ap()
    wo = nc.dram_tensor("wo", [256, DIM], BF16, kind="ExternalInput").ap()
    # Partials are stored bf16 (halves store DMA; host sums in f32 - adds
    # ~0.1% rms to one of four partials, well inside the error budget).
    out = nc.dram_tensor("out", [DIM, NSEQ], BF16, kind="ExternalOutput").ap()
    if DEBUG_DUMP:
        dbg_qbT = nc.dram_tensor("dbg_qbT", [128, 2, NSEQ], BF16,
                                 kind="ExternalOutput").ap()
        dbg_kbT = nc.dram_tensor("dbg_kbT", [128, NSEQ], BF16,
                                 kind="ExternalOutput").ap()
        dbg_vn = nc.dram_tensor("dbg_vn", [128, NSEQ // 128, 65], BF16,
                                kind="ExternalOutput").ap()
        dbg_aoutT = nc.dram_tensor("dbg_aoutT", [128, 2, NSEQ], mybir.dt.float32,
                                   kind="ExternalOutput").ap()
        dbg_xT = nc.dram_tensor("dbg_xT", [128, DIM // 128, NSEQ], BF16,
                                kind="ExternalOutput").ap()
        dbg_xT2 = nc.dram_tensor("dbg_xT2", [128, DIM // 128, NSEQ], BF16,
                                 kind="ExternalOutput").ap()

    with tile.TileContext(nc) as tc, ExitStack() as ctx:
        sb = ctx.enter_context(tc.tile_pool(name="sb", bufs=1))

        wqkv_sb = sb.tile([128, DKT, 384], BF16)
        wq_sb = wqkv_sb[:, :, 0:256]
        wkv_sb = wqkv_sb[:, :, 256:384]
        # wo is bf16: stationary dtype doesn't affect matmul cost, and an
        # F32R DRAM tensor (or bitcast AP) corrupts in-flight xbar-transpose
        # descriptors, so plain bf16 is both faster to load and safe.
        wo_sb = sb.tile([128, 2, DIM], BF16)
        ident = sb.tile([128, 128], F32)
        identb = sb.tile([128, 128], BF16)
        identr = sb.tile([128, 128], F32R)
        warm_in = sb.tile([128, 1], F32)
        warm = sb.tile([128, 1], F32)

        xT = sb.tile([128, DKT, NSEQ], BF16)
        qbT = sb.tile([128, 2, NSEQ], BF16)
        kbT = sb.tile([128, NSEQ], BF16)
        vn = sb.tile([128, KT, D + 1], BF16)
        aoutT = sb.tile([128, 2, NSEQ], BF16)

        vtp = ctx.enter_context(tc.tile_pool(name="vtp", bufs=2))
        ptp = ctx.enter_context(tc.tile_pool(name="ptp", bufs=16))
        rrp = ctx.enter_context(tc.tile_pool(name="rrp", bufs=2))
        aop = ctx.enter_context(tc.tile_pool(name="aop", bufs=2))
        outp = ctx.enter_context(tc.tile_pool(name="outp", bufs=4))
        ps_sc = ctx.enter_context(tc.tile_pool(name="ps_sc", bufs=2, space="PSUM"))
        ps_pv = ctx.enter_context(tc.tile_pool(name="ps_pv", bufs=2, space="PSUM"))
        ps_pj = ctx.enter_context(tc.tile_pool(name="ps_pj", bufs=1, space="PSUM"))
        ps_su = ctx.enter_context(tc.tile_pool(name="ps_su", bufs=1, space="PSUM"))

        state = {}     # qc -> [hp0_tile, hp1_tile] each [128, 2, 4, 64]
        sums = {}      # qc -> [128, 16] psum tile (cols h*4+i)
        rr_tiles = {}
        pending_pv = []
        fillq = deque()

        def fill(n=1):
            for _ in range(n):
                if not fillq:
                    return
                fillq.popleft()()

        # ---------------- work units ----------------
        def unit_pkv(sg, dup_eng):
            def run():
                pkv = ps_pj.tile([128, 512], F32, tag="pj", name=f"pkv{sg}")
                for d in range(DKT):
                    nc.tensor.matmul(pkv[:, 0:512], wkv_sb[:, d, :],
                                     xT[:, d, ds(sg * 512, 512)],
                                     start=(d == 0), stop=(d == DKT - 1))
                nc.vector.tensor_copy(kbT[ds(0, 64), ds(sg * 512, 512)],
                                      pkv[ds(0, 64), 0:512])
                eng = nc.scalar if (dup_eng == "act" and ACT_DUP) else nc.sync
                eng.dma_start(out=kbT[ds(64, 64), ds(sg * 512, 512)],
                              in_=kbT[ds(0, 64), ds(sg * 512, 512)])
                vtmp = vtp.tile([64, 512], BF16, tag="vt", name=f"vt{sg}")
                nc.vector.tensor_copy(vtmp, pkv[ds(64, 64), 0:512])
                vtmp_tiles[sg] = vtmp
            return run

        def unit_ptv(sg):
            def run():
                ptv = ps_pj.tile([128, 1024], BF16, tag="pj", name=f"ptv{sg}")
                for i in range(4):
                    nc.tensor.transpose(ptv[:, ds(i * D, D)],
                                        vtmp_tiles[sg][:, ts(i, 128)],
                                        identb[0:64, 0:64])
                nc.vector.tensor_copy(vn[:, ds(sg * 4, 4), 0:D], ptv[:, 0:4 * D])
            return run

        def unit_qt(qc, p, hlf=None, pool=None):
            def run():
                if hlf is None:
                    qw, off = 512, 0
                else:
                    qw, off = 256, hlf * 256
                p_ = pool if pool is not None else ps_pj
                tag = "pj" if p_ is ps_pj else "sc"
                shape = [128, 512] if p_ is ps_pj else [128, 1024]
                pq = p_.tile(shape, F32, tag=tag, name=f"pq{qc}_{p}_{off}")
                for d in range(DKT):
                    nc.tensor.matmul(pq[:, 0:qw], wq_sb[:, d, ts(p, 128)],
                                     xT[:, d, ds(qc * 512 + off, qw)],
                                     start=(d == 0), stop=(d == DKT - 1))
                if ACT_QBT:
                    nc.scalar.activation(out=qbT[:, p, ds(qc * 512 + off, qw)],
                                         in_=pq[:, 0:qw], func=COPYF, scale=1.0)
                else:
                    nc.vector.tensor_copy(qbT[:, p, ds(qc * 512 + off, qw)],
                                          pq[:, 0:qw])
            return run

        def unit_po_pair(qc, op, eng="act"):
            """O-projection for od = 2*op, 2*op+1 through a [128,1024] sc tile."""
            def run():
                po = ps_sc.tile([128, 1024], F32, tag="sc", name=f"po{qc}_{op}")
                for k in range(2):
                    od = 2 * op + k
                    nc.tensor.matmul(po[:, ds(k * 512, 512)],
                                     wo_sb[:, 0, ts(od, 128)],
                                     aoutT[:, 0, ds(qc * 512, 512)],
                                     start=True, stop=False)
                    nc.tensor.matmul(po[:, ds(k * 512, 512)],
                                     wo_sb[:, 1, ts(od, 128)],
                                     aoutT[:, 1, ds(qc * 512, 512)],
                                     start=False, stop=True)
                ot = outp.tile([128, 2, 512], BF16, tag="ot", name=f"ot{qc}_{op}")
                if eng == "act":
                    nc.scalar.activation(out=ot, in_=po, func=COPYF, scale=1.0)
                else:
                    nc.vector.tensor_copy(ot, po)
                nc.sync.dma_start(
                    out=out[ds(op * 256, 256), ds(qc * 512, 512)].rearrange(
                        "(t p) m -> p t m", p=128),
                    in_=ot)
            return run

        # ---------------- attention ----------------
        def flush_pv_one(qc_, j_, h_, pt_):
            # Flush order within a quantum is h0, h1, h3, h2 (see
            # emit_quanta), so the first series to touch the hp1 state bank
            # is hh==1. start=True zeroes the whole 2KB PSUM bank: only that
            # first series may set it.
            hp, hh = h_ // 2, h_ % 2
            first_hh = 1 if hp == 1 else 0
            for t in range(2):
                kt = 2 * j_ + t
                for i in range(4):
                    stn = pt_[:, ds(t * 512 + i * 128, 128)]
                    nc.tensor.matmul(state[qc_][hp][:, hh, i, :], stn,
                                     vn[:, kt, 0:D],
                                     start=(kt == 0 and i == 0
                                            and hh == first_hh),
                                     stop=(kt == KT - 1),
                                     skip_group_check=True)
                    nc.tensor.matmul(sums[qc_][:, ds(h_ * 4 + i, 1)], stn,
                                     vn[:, kt, D:D + 1],
                                     start=(kt == 0 and i == 0 and h_ == 0),
                                     stop=(kt == KT - 1),
                                     skip_group_check=True)

        def flush_pv():
            for (qc_, j_, h_, pt_) in pending_pv:
                flush_pv_one(qc_, j_, h_, pt_)
            pending_pv.clear()

        def exp_engine(qc, j, h):
            # Strict engine alternation: psc buffer k is reused two tiles
            # later, so exp(k) gates psc(k+2). With h0/h2 on DVE and h1/h3 on
            # Act, same-engine exps are two buffer-spacings apart and never
            # queue behind each other.
            return "dve" if h in (0, 2) else "act"

        def emit_quanta(qc, j, mid_fills=(3,)):
            # The previous quantum's P@V is interleaved per-head between this
            # quantum's score matmuls so PE has work inside every exp-wait.
            prev = list(pending_pv)
            pending_pv.clear()
            for h in range(4):
                p, i = h // 2, h % 2
                psc = ps_sc.tile([128, 1024], F32, tag="sc", name=f"psc{qc}_{j}_{h}")
                for t in range(2):
                    kt = 2 * j + t
                    nc.tensor.matmul(psc[:, ds(t * 512, 512)],
                                     kbT[ds(i * 64, 64), ts(kt, 128)],
                                     qbT[ds(i * 64, 64), p, ds(qc * 512, 512)],
                                     start=True, stop=True)
                pt = ptp.tile([128, 1024], BF16, tag="pt", name=f"pt{qc}_{j}_{h}")
                if not USE_SCHRAUDOLPH or exp_engine(qc, j, h) == "act":
                    nc.scalar.activation(out=pt, in_=psc, func=EXPF, scale=SCALE)
                else:
                    nc.vector.tensor_scalar(
                        out=pt.bitcast(I16), in0=psc, scalar1=A_SCH,
                        scalar2=B_SCH, op0=MULT, op1=ADD)
                pending_pv.append((qc, j, h, pt))
                # flush order h0, h1, h3, h2: the DVE-run exps (h0/h2) gate
                # the two-buffer psc rotation, so the even-head score matmuls
                # get extra PE work in front of them.
                if prev:
                    if h < 2:
                        flush_pv_one(*prev[h])
                    elif h == 2:
                        flush_pv_one(*prev[2])
                if h == 1 and prev:
                    flush_pv_one(*prev[3])
                if h in mid_fills:
                    fill(1)

        def alloc_state(qc):
            state[qc] = [
                ps_pv.tile([128, 2, 4, D], F32, tag="pv", name=f"pv{qc}_{hp}")
                for hp in range(2)
            ]
            sums[qc] = ps_su.tile([128, 16], F32, tag="su", name=f"su{qc}")

        ao_tiles = {}

        def norm_muls(qc, hps):
            """Per-head scaling: one fused DVE op per head, out = state * rr
            broadcast along i,d. ao layout [128, i, h, d] keeps each
            pat-transpose input slice contiguous; per-head output stays 3D."""
            ao = ao_tiles[qc]
            rr = rr_tiles[qc]
            for hp in hps:
                for hh in range(2):
                    h = 2 * hp + hh
                    rr_b = rr[:, ds(4 * h, 4)].unsqueeze(-1).broadcast_to(
                        [128, 4, D])
                    nc.vector.scalar_tensor_tensor(
                        out=ao[:, :, h, :], in0=state[qc][hp][:, hh, :, :],
                        scalar=1.0, in1=rr_b, op0=MULT, op1=MULT)

        def emit_norm_muls(qc):
            """Phase A: reciprocal + hp0 scaling; hp1's muls ride the j1
            fill to smooth DVE's qc-boundary load."""
            rr = rrp.tile([128, 16], F32, tag="rr", name=f"rr{qc}")
            nc.vector.reciprocal(out=rr, in_=sums[qc])
            rr_tiles[qc] = rr
            ao_tiles[qc] = aop.tile([128, 4, 4, D], F32R, tag="ao",
                                    name=f"ao{qc}")
            norm_muls(qc, (0, 1))

        def unit_patT(qc, hps=(0, 1), use_act=False, pre_muls=()):
            """Phase B: PE transposes of the normalized heads + aoutT copies.
            Dispatched a quantum after phase A so PE never chases the muls."""
            def run():
                norm_muls(qc, pre_muls)
                ao = ao_tiles[qc]
                for hp in hps:
                    pat = ps_pj.tile([128, 512], F32R, tag="pj",
                                     name=f"pat{qc}_{hp}")
                    for i in range(4):
                        nc.tensor.transpose(pat[:, ds(i * 128, 128)],
                                            ao[:, i, ds(2 * hp, 2), :], identr)
                    # Act takes both aoutT copies: DVE is the exp
                    # bottleneck at qc boundaries.
                    nc.scalar.activation(
                        out=aoutT[:, hp, ds(qc * 512, 512)],
                        in_=pat, func=COPYF, scale=1.0)
            return run

        # ---------------- schedule ----------------
        vtmp_tiles = {}

        # DMA queue order sets the data-arrival schedule. Weight loads are
        # interleaved between the first x-transpose blocks so nothing waits
        # a full 8-deep DMA batch; x chunks 2-3 are issued as fill units
        # during qc0 so later DMAs (kbT dup, stores) don't queue behind them.
        def dma_xt(sp):
            # one transpose DMA per (chunk-pair, 128-col block): 16 DMAs
            # total instead of 32 - HWDGE issue overhead (~0.65us per DMA)
            # paces x delivery at startup.
            for t in range(DKT):
                nc.sync.dma_start_transpose(
                    out=xT[:, t, ds(sp * 1024, 1024)],
                    in_=x[ds(sp * 1024, 1024), ds(t * 128, 128)])

        nc.sync.dma_start(out=wqkv_sb,
                          in_=wqkv.rearrange("(t p) m -> p t m", p=128))
        dma_xt(0)
        dma_xt(1)
        nc.sync.dma_start(out=wo_sb,
                          in_=wo.rearrange("(t p) m -> p t m", p=128))

        make_identity(nc, ident)
        nc.vector.tensor_copy(identb, ident)
        nc.vector.tensor_copy(identr, ident)
        nc.vector.memset(vn, 1.0)
        nc.vector.memset(warm_in, 1.0)
        # preload the exp table set off the critical path
        nc.scalar.activation(out=warm, in_=warm_in, func=EXPF, scale=1.0)

        if DEBUG_DUMP:
            nc.sync.dma_start(out=dbg_xT2, in_=xT)
        unit_pkv(0, "act")()
        unit_qt(0, 0, pool=ps_sc)()
        unit_qt(0, 1, pool=ps_sc)()
        unit_ptv(0)()
        unit_pkv(1, "act")()
        unit_ptv(1)()
        alloc_state(0)

        # qc 0: kbT chunk s is consumed from j=2s; chunks 2-3 land mid-loop.
        fills_at = {2: [unit_pkv(2, "sync")], 3: [unit_ptv(2)],
                    4: [unit_pkv(3, "sync")],
                    5: [unit_ptv(3), unit_qt(1, 0, 0)],
                    6: [unit_qt(1, 0, 1), unit_qt(1, 1, 0)],
                    7: [unit_qt(1, 1, 1)]}
        for j in range(KT // 2):
            fillq.extend(fills_at.get(j, []))
            emit_quanta(0, j)
            fill(1)
        for qc in range(1, QC):
            emit_quanta(qc, 0)       # flushes (qc-1, 7) into state[qc-1]
            emit_norm_muls(qc - 1)
            alloc_state(qc)
            fills_qc = {1: [unit_patT(qc - 1)],
                        2: [unit_po_pair(qc - 1, 0, "act")],
                        3: [unit_po_pair(qc - 1, 1, "act")],
                        4: [unit_po_pair(qc - 1, 2, "act")],
                        5: [unit_po_pair(qc - 1, 3, "act")]}
            if qc + 1 < QC:
                fills_qc[6] = [unit_qt(qc + 1, 0, 0), unit_qt(qc + 1, 0, 1)]
                fills_qc[7] = [unit_qt(qc + 1, 1, 0), unit_qt(qc + 1, 1, 1)]
            for j in range(1, KT // 2):
                fillq.extend(fills_qc.get(j, []))
                emit_quanta(qc, j)
                fill(1)
        flush_pv()
        emit_norm_muls(QC - 1)
        unit_patT(QC - 1, use_act=True)()  # both head pairs
        fill(len(fillq))
        for op in range(4):
            unit_po_pair(QC - 1, op, "act" if op % 2 == 0 else "dve")()
        if DEBUG_DUMP:
            nc.sync.dma_start(out=dbg_qbT, in_=qbT)
            nc.sync.dma_start(out=dbg_kbT, in_=kbT)
            nc.sync.dma_start(out=dbg_vn, in_=vn)
            nc.sync.dma_start(out=dbg_aoutT, in_=aoutT.bitcast(F32))
            nc.sync.dma_start(out=dbg_xT, in_=xT)

    nc.compile()
    return nc


_CACHE = {}


def _get_nc(NSEQ):
    if NSEQ not in _CACHE:
        _CACHE[NSEQ] = build_nc(NSEQ)
    return _CACHE[NSEQ]


def kernel(x, Wq, Wk, Wv, Wo, bo):
    """Full-input entry point: shard over 8 cores, run, gather."""
    x, Wq, Wk, Wv, Wo, bo = (np.asarray(a, np.float32) for a in (x, Wq, Wk, Wv, Wo, bo))
    B, N, C = x.shape
    nc = _get_nc(N)
    in_maps = []
    for c in range(8):
        b, g = c // 4, c % 4
        in_maps.append({
            "x": np.ascontiguousarray(x[b]).astype(BF16_NP),
            "wqkv": np.ascontiguousarray(np.concatenate(
                [Wq[:, g * 256:(g + 1) * 256],
                 Wk[:, g * D:(g + 1) * D], Wv[:, g * D:(g + 1) * D]],
                axis=1)).astype(BF16_NP),
            "wo": np.ascontiguousarray(
                Wo[g * 256:(g + 1) * 256, :]).astype(BF16_NP),
        })
    res = bass_utils.run_bass_kernel_spmd(nc, in_maps, core_ids=list(range(8)))
    outs = [res.results[c]["out"] for c in range(8)]
    full = np.empty((B, N, C), np.float32)
    for b in range(B):
        acc = outs[4 * b].astype(np.float32)
        for g in range(1, 4):
            acc = acc + outs[4 * b + g]
        full[b] = acc.T + bo[None, :]
    return full


# revision 58
# speedup vs baseline: 1.0839x; 1.0010x over previous
"""GQA attention kernel for Trainium2 (Bass/Tile), 8-core SPMD.

Problem: B=2, N=2048, DIM=1024, 16 query heads / 4 KV heads, head_dim=64, fp32.
Sharding: core c = (batch b=c//4, kv-group g=c%4). Each core computes its
group's 4 query heads + 1 shared KV head over the full sequence, and a partial
output projection (its 256 rows of Wo). Host sums the 4 group partials per
batch and adds the bias.

Engine plan (per-core busy targets, cost model). GPSIMD cannot touch PSUM on
TRN2, and every elementwise op here reads PSUM, so all of it splits between
Act and DVE:
  PE ~119us: scores (S^T, 128-key tiles x 512-query moving), P@V with P^T
    stationary, Q/KV/O projections, small V and aout transposes.
  Act ~100us: 80 of 128 exp tiles ([128,1024] PSUM->SBUF bf16) + the 16
    O-projection staging copies.
  DVE ~86us: 48 exp tiles via a single fused tensor_scalar op - Schraudolph
    bf16 exp: int16(out) = round(s*A + B) bit-cast to bf16 (max rel err ~4%
    on ~37% of keys -> ~1% on the final output) - plus projection-result
    copies, softmax normalization, reciprocal.
  DMA ~55us: x is loaded pre-transposed via dma_start_transpose (xbar,
    14ns/16x128 tile) straight into xT bf16 - no on-chip transpose pass.

PSUM (8 banks): scores 2x[128,1024]f32 (4) + P@V accumulators 2x[128,2,4,64]
(2) + projection staging [128,512] (1) + sum-of-exp [128,16] (1). Sum-of-exp
rides the PV matmuls via a ones column in vn. O-projection goes through the
score pool as [128,1024] od-pairs; Q-projection fills through the staging
bank at most once per quantum so bank-reuse handoffs stay off PE's critical
path.
"""

import sys

if "/opt/trn_rl_repo" not in sys.path:
    sys.path.insert(0, "/opt/trn_rl_repo")

from collections import deque
from contextlib import ExitStack

import ml_dtypes
import numpy as np

BF16_NP = ml_dtypes.bfloat16

import concourse.bass as bass
import concourse.mybir as mybir
import concourse.tile as tile
from concourse import bacc, bass_utils
from concourse.bass import ds, ts
from concourse.masks import make_identity

F32 = mybir.dt.float32
F32R = mybir.dt.float32r
BF16 = mybir.dt.bfloat16
I16 = mybir.dt.int16
EXPF = mybir.ActivationFunctionType.Exp
COPYF = mybir.ActivationFunctionType.Copy
MULT = mybir.AluOpType.mult
ADD = mybir.AluOpType.add

DIM = 1024
D = 64  # head dim
SCALE = D ** -0.5

# Schraudolph bf16 exp constants: bf16_bits(exp(s*SCALE)) ~= round(s*A + B)
A_SCH = SCALE * 128.0 / np.log(2.0)
B_SCH = 16256.0 - 6.5  # C=6.5 calibrated for round-to-nearest f32->int16


USE_SCHRAUDOLPH = True
USE_STT_NORM = True
DEBUG_DUMP = False
ACT_DUP = True
ACT_QBT = True


def build_nc(NSEQ=2048):
    KT = NSEQ // 128   # key tiles
    QC = NSEQ // 512   # query chunks of 512
    DKT = DIM // 128   # contraction tiles for projections

    nc = bacc.Bacc("TRN2", target_bir_lowering=False, debug=False)
    x = nc.dram_tensor("x", [NSEQ, DIM], BF16, kind="ExternalInput").ap()
    wqkv = nc.dram_tensor("wqkv", [DIM, 384], BF16, kind="ExternalInput").ap()
    wo = nc.dram_tensor("wo", [256, DIM], BF16, kind="ExternalInput").ap()
    # Partials are stored bf16 (halves store DMA; host sums in f32 - adds
    # ~0.1% rms to one of four partials, well inside the error budget).
    out = nc.dram_tensor("out", [DIM, NSEQ], BF16, kind="ExternalOutput").ap()
    if DEBUG_DUMP:
        dbg_qbT = nc.dram_tensor("dbg_qbT", [128, 2, NSEQ], BF16,
                                 kind="ExternalOutput").ap()
        dbg_kbT = nc.dram_tensor("dbg_kbT", [128, NSEQ], BF16,
                                 kind="ExternalOutput").ap()
        dbg_vn = nc.dram_tensor("dbg_vn", [128, NSEQ // 128, 65], BF16,
                                kind="ExternalOutput").ap()
        dbg_aoutT = nc.dram_tensor("dbg_aoutT", [128, 2, NSEQ], mybir.dt.float32,
                                   kind="ExternalOutput").ap()
        dbg_xT = nc.dram_tensor("dbg_xT", [128, DIM // 128, NSEQ], BF16,
                                kind="ExternalOutput").ap()
        dbg_xT2 = nc.dram_tensor("dbg_xT2", [128, DIM // 128, NSEQ], BF16,
                                 kind="ExternalOutput").ap()

    with tile.TileContext(nc) as tc, ExitStack() as ctx:
        sb = ctx.enter_context(tc.tile_pool(name="sb", bufs=1))

        wqkv_sb = sb.tile([128, DKT, 384], BF16)
        wq_sb = wqkv_sb[:, :, 0:256]
        wkv_sb = wqkv_sb[:, :, 256:384]
        # wo is bf16: stationary dtype doesn't affect matmul cost, and an
        # F32R DRAM tensor (or bitcast AP) corrupts in-flight xbar-transpose
        # descriptors, so plain bf16 is both faster to load and safe.
        wo_sb = sb.tile([128, 2, DIM], BF16)
        ident = sb.tile([128, 128], F32)
        identb = sb.tile([128, 128], BF16)
        identr = sb.tile([128, 128], F32R)
        warm_in = sb.tile([128, 1], F32)
        warm = sb.tile([128, 1], F32)

        xT = sb.tile([128, DKT, NSEQ], BF16)
        qbT = sb.tile([128, 2, NSEQ], BF16)
        kbT = sb.tile([128, NSEQ], BF16)
        vn = sb.tile([128, KT, D + 1], BF16)
        aoutT = sb.tile([128, 2, NSEQ], BF16)

        vtp = ctx.enter_context(tc.tile_pool(name="vtp", bufs=2))
        ptp = ctx.enter_context(tc.tile_pool(name="ptp", bufs=16))
        rrp = ctx.enter_context(tc.tile_pool(name="rrp", bufs=2))
        aop = ctx.enter_context(tc.tile_pool(name="aop", bufs=2))
        outp = ctx.enter_context(tc.tile_pool(name="outp", bufs=4))
        ps_sc = ctx.enter_context(tc.tile_pool(name="ps_sc", bufs=2, space="PSUM"))
        ps_pv = ctx.enter_context(tc.tile_pool(name="ps_pv", bufs=2, space="PSUM"))
        ps_pj = ctx.enter_context(tc.tile_pool(name="ps_pj", bufs=1, space="PSUM"))
        ps_su = ctx.enter_context(tc.tile_pool(name="ps_su", bufs=1, space="PSUM"))

        state = {}     # qc -> [hp0_tile, hp1_tile] each [128, 2, 4, 64]
        sums = {}      # qc -> [128, 16] psum tile (cols h*4+i)
        rr_tiles = {}
        pending_pv = []
        fillq = deque()

        def fill(n=1):
            for _ in range(n):
                if not fillq:
                    return
                fillq.popleft()()

        # ---------------- work units ----------------
        def unit_pkv(sg, dup_eng):
            def run():
                pkv = ps_pj.tile([128, 512], F32, tag="pj", name=f"pkv{sg}")
                for d in range(DKT):
                    nc.tensor.matmul(pkv[:, 0:512], wkv_sb[:, d, :],
                                     xT[:, d, ds(sg * 512, 512)],
                                     start=(d == 0), stop=(d == DKT - 1))
                nc.vector.tensor_copy(kbT[ds(0, 64), ds(sg * 512, 512)],
                                      pkv[ds(0, 64), 0:512])
                eng = nc.scalar if (dup_eng == "act" and ACT_DUP) else nc.sync
                eng.dma_start(out=kbT[ds(64, 64), ds(sg * 512, 512)],
                              in_=kbT[ds(0, 64), ds(sg * 512, 512)])
                vtmp = vtp.tile([64, 512], BF16, tag="vt", name=f"vt{sg}")
                nc.vector.tensor_copy(vtmp, pkv[ds(64, 64), 0:512])
                vtmp_tiles[sg] = vtmp
            return run

        def unit_ptv(sg):
            def run():
                ptv = ps_pj.tile([128, 1024], BF16, tag="pj", name=f"ptv{sg}")
                for i in range(4):
                    nc.tensor.transpose(ptv[:, ds(i * D, D)],
                                        vtmp_tiles[sg][:, ts(i, 128)],
                                        identb[0:64, 0:64])
                nc.vector.tensor_copy(vn[:, ds(sg * 4, 4), 0:D], ptv[:, 0:4 * D])
            return run

        def unit_qt(qc, p, hlf=None, pool=None):
            def run():
                if hlf is None:
                    qw, off = 512, 0
                else:
                    qw, off = 256, hlf * 256
                p_ = pool if pool is not None else ps_pj
                tag = "pj" if p_ is ps_pj else "sc"
                shape = [128, 512] if p_ is ps_pj else [128, 1024]
                pq = p_.tile(shape, F32, tag=tag, name=f"pq{qc}_{p}_{off}")
                for d in range(DKT):
                    nc.tensor.matmul(pq[:, 0:qw], wq_sb[:, d, ts(p, 128)],
                                     xT[:, d, ds(qc * 512 + off, qw)],
                                     start=(d == 0), stop=(d == DKT - 1))
                if ACT_QBT:
                    nc.scalar.activation(out=qbT[:, p, ds(qc * 512 + off, qw)],
                                         in_=pq[:, 0:qw], func=COPYF, scale=1.0)
                else:
                    nc.vector.tensor_copy(qbT[:, p, ds(qc * 512 + off, qw)],
                                          pq[:, 0:qw])
            return run

        def unit_po_pair(qc, op, eng="act"):
            """O-projection for od = 2*op, 2*op+1 through a [128,1024] sc tile."""
            def run():
                po = ps_sc.tile([128, 1024], F32, tag="sc", name=f"po{qc}_{op}")
                for k in range(2):
                    od = 2 * op + k
                    nc.tensor.matmul(po[:, ds(k * 512, 512)],
                                     wo_sb[:, 0, ts(od, 128)],
                                     aoutT[:, 0, ds(qc * 512, 512)],
                                     start=True, stop=False)
                    nc.tensor.matmul(po[:, ds(k * 512, 512)],
                                     wo_sb[:, 1, ts(od, 128)],
                                     aoutT[:, 1, ds(qc * 512, 512)],
                                     start=False, stop=True)
                ot = outp.tile([128, 2, 512], BF16, tag="ot", name=f"ot{qc}_{op}")
                if eng == "act":
                    nc.scalar.activation(out=ot, in_=po, func=COPYF, scale=1.0)
                else:
                    nc.vector.tensor_copy(ot, po)
                nc.sync.dma_start(
                    out=out[ds(op * 256, 256), ds(qc * 512, 512)].rearrange(
                        "(t p) m -> p t m", p=128),
                    in_=ot)
            return run

        # ---------------- attention ----------------
        def flush_pv_one(qc_, j_, h_, pt_):
            # Flush order within a quantum is h0, h1, h3, h2 (see
            # emit_quanta), so the first series to touch the hp1 state bank
            # is hh==1. start=True zeroes the whole 2KB PSUM bank: only that
            # first series may set it.
            hp, hh = h_ // 2, h_ % 2
            first_hh = 1 if hp == 1 else 0
            for t in range(2):
                kt = 2 * j_ + t
                for i in range(4):
                    stn = pt_[:, ds(t * 512 + i * 128, 128)]
                    nc.tensor.matmul(state[qc_][hp][:, hh, i, :], stn,
                                     vn[:, kt, 0:D],
                                     start=(kt == 0 and i == 0
                                            and hh == first_hh),
                                     stop=(kt == KT - 1),
                                     skip_group_check=True)
                    nc.tensor.matmul(sums[qc_][:, ds(h_ * 4 + i, 1)], stn,
                                     vn[:, kt, D:D + 1],
                                     start=(kt == 0 and i == 0 and h_ == 0),
                                     stop=(kt == KT - 1),
                                     skip_group_check=True)

        def flush_pv():
            for (qc_, j_, h_, pt_) in pending_pv:
                flush_pv_one(qc_, j_, h_, pt_)
            pending_pv.clear()

        def exp_engine(qc, j, h):
            # Strict engine alternation: psc buffer k is reused two tiles
            # later, so exp(k) gates psc(k+2). With h0/h2 on DVE and h1/h3 on
            # Act, same-engine exps are two buffer-spacings apart and never
            # queue behind each other.
            return "dve" if h in (0, 2) else "act"

        def emit_quanta(qc, j, mid_fills=(3,)):
            # The previous quantum's P@V is interleaved per-head between this
            # quantum's score matmuls so PE has work inside every exp-wait.
            prev = list(pending_pv)
            pending_pv.clear()
            for h in range(4):
                p, i = h // 2, h % 2
                psc = ps_sc.tile([128, 1024], F32, tag="sc", name=f"psc{qc}_{j}_{h}")
                for t in range(2):
                    kt = 2 * j + t
                    nc.tensor.matmul(psc[:, ds(t * 512, 512)],
                                     kbT[ds(i * 64, 64), ts(kt, 128)],
                                     qbT[ds(i * 64, 64), p, ds(qc * 512, 512)],
                                     start=True, stop=True)
                pt = ptp.tile([128, 1024], BF16, tag="pt", name=f"pt{qc}_{j}_{h}")
                if not USE_SCHRAUDOLPH or exp_engine(qc, j, h) == "act":
                    nc.scalar.activation(out=pt, in_=psc, func=EXPF, scale=SCALE)
                else:
                    nc.vector.tensor_scalar(
                        out=pt.bitcast(I16), in0=psc, scalar1=A_SCH,
                        scalar2=B_SCH, op0=MULT, op1=ADD)
                pending_pv.append((qc, j, h, pt))
                # flush order h0, h1, h3, h2: the DVE-run exps (h0/h2) gate
                # the two-buffer psc rotation, so the even-head score matmuls
                # get extra PE work in front of them.
                if prev:
                    if h < 2:
                        flush_pv_one(*prev[h])
                    elif h == 2:
                        flush_pv_one(*prev[2])
                if h == 1 and prev:
                    flush_pv_one(*prev[3])
                if h in mid_fills:
                    fill(1)

        def alloc_state(qc):
            state[qc] = [
                ps_pv.tile([128, 2, 4, D], F32, tag="pv", name=f"pv{qc}_{hp}")
                for hp in range(2)
            ]
            sums[qc] = ps_su.tile([128, 16], F32, tag="su", name=f"su{qc}")

        ao_tiles = {}

        def norm_muls(qc, hps):
            """Per-head scaling: one fused DVE op per head, out = state * rr
            broadcast along i,d. ao layout [128, i, h, d] keeps each
            pat-transpose input slice contiguous; per-head output stays 3D."""
            ao = ao_tiles[qc]
            rr = rr_tiles[qc]
            for hp in hps:
                for hh in range(2):
                    h = 2 * hp + hh
                    rr_b = rr[:, ds(4 * h, 4)].unsqueeze(-1).broadcast_to(
                        [128, 4, D])
                    nc.vector.scalar_tensor_tensor(
                        out=ao[:, :, h, :], in0=state[qc][hp][:, hh, :, :],
                        scalar=1.0, in1=rr_b, op0=MULT, op1=MULT)

        def emit_norm_muls(qc):
            """Phase A: reciprocal + hp0 scaling; hp1's muls ride the j1
            fill to smooth DVE's qc-boundary load."""
            rr = rrp.tile([128, 16], F32, tag="rr", name=f"rr{qc}")
            nc.vector.reciprocal(out=rr, in_=sums[qc])
            rr_tiles[qc] = rr
            ao_tiles[qc] = aop.tile([128, 4, 4, D], BF16, tag="ao",
                                    name=f"ao{qc}")
            norm_muls(qc, (0, 1))

        def unit_patT(qc, hps=(0, 1), use_act=False, pre_muls=()):
            """Phase B: PE transposes of the normalized heads + aoutT copies.
            Dispatched a quantum after phase A so PE never chases the muls."""
            def run():
                norm_muls(qc, pre_muls)
                ao = ao_tiles[qc]
                for hp in hps:
                    pat = ps_pj.tile([128, 512], BF16, tag="pj",
                                     name=f"pat{qc}_{hp}")
                    for i in range(4):
                        nc.tensor.transpose(pat[:, ds(i * 128, 128)],
                                            ao[:, i, ds(2 * hp, 2), :], identb)
                    # Act takes both aoutT copies: DVE is the exp
                    # bottleneck at qc boundaries.
                    nc.scalar.activation(
                        out=aoutT[:, hp, ds(qc * 512, 512)],
                        in_=pat, func=COPYF, scale=1.0)
            return run

        # ---------------- schedule ----------------
        vtmp_tiles = {}

        # DMA queue order sets the data-arrival schedule. Weight loads are
        # interleaved between the first x-transpose blocks so nothing waits
        # a full 8-deep DMA batch; x chunks 2-3 are issued as fill units
        # during qc0 so later DMAs (kbT dup, stores) don't queue behind them.
        def dma_xt(sp):
            # one transpose DMA per (chunk-pair, 128-col block): 16 DMAs
            # total instead of 32 - HWDGE issue overhead (~0.65us per DMA)
            # paces x delivery at startup.
            for t in range(DKT):
                nc.sync.dma_start_transpose(
                    out=xT[:, t, ds(sp * 1024, 1024)],
                    in_=x[ds(sp * 1024, 1024), ds(t * 128, 128)])

        nc.sync.dma_start(out=wqkv_sb,
                          in_=wqkv.rearrange("(t p) m -> p t m", p=128))
        dma_xt(0)
        dma_xt(1)
        nc.sync.dma_start(out=wo_sb,
                          in_=wo.rearrange("(t p) m -> p t m", p=128))

        make_identity(nc, ident)
        nc.vector.tensor_copy(identb, ident)
        nc.vector.tensor_copy(identr, ident)
        nc.vector.memset(vn, 1.0)
        nc.vector.memset(warm_in, 1.0)
        # preload the exp table set off the critical path
        nc.scalar.activation(out=warm, in_=warm_in, func=EXPF, scale=1.0)

        if DEBUG_DUMP:
            nc.sync.dma_start(out=dbg_xT2, in_=xT)
        unit_pkv(0, "act")()
        unit_qt(0, 0, pool=ps_sc)()
        unit_qt(0, 1, pool=ps_sc)()
        unit_ptv(0)()
        unit_pkv(1, "act")()
        unit_ptv(1)()
        alloc_state(0)

        # qc 0: kbT chunk s is consumed from j=2s; chunks 2-3 land mid-loop.
        fills_at = {2: [unit_pkv(2, "sync")], 3: [unit_ptv(2)],
                    4: [unit_pkv(3, "sync")],
                    5: [unit_ptv(3), unit_qt(1, 0, 0)],
                    6: [unit_qt(1, 0, 1), unit_qt(1, 1, 0)],
                    7: [unit_qt(1, 1, 1)]}
        for j in range(KT // 2):
            fillq.extend(fills_at.get(j, []))
            emit_quanta(0, j)
            fill(1)
        for qc in range(1, QC):
            emit_quanta(qc, 0)       # flushes (qc-1, 7) into state[qc-1]
            emit_norm_muls(qc - 1)
            alloc_state(qc)
            fills_qc = {1: [unit_patT(qc - 1)],
                        2: [unit_po_pair(qc - 1, 0, "act")],
                        3: [unit_po_pair(qc - 1, 1, "act")],
                        4: [unit_po_pair(qc - 1, 2, "act")],
                        5: [unit_po_pair(qc - 1, 3, "act")]}
            if qc + 1 < QC:
                fills_qc[6] = [unit_qt(qc + 1, 0, 0), unit_qt(qc + 1, 0, 1)]
                fills_qc[7] = [unit_qt(qc + 1, 1, 0), unit_qt(qc + 1, 1, 1)]
            for j in range(1, KT // 2):
                fillq.extend(fills_qc.get(j, []))
                emit_quanta(qc, j)
                fill(1)
        flush_pv()
        emit_norm_muls(QC - 1)
        unit_patT(QC - 1, use_act=True)()  # both head pairs
        fill(len(fillq))
        for op in range(4):
            unit_po_pair(QC - 1, op, "act" if op % 2 == 0 else "dve")()
        if DEBUG_DUMP:
            nc.sync.dma_start(out=dbg_qbT, in_=qbT)
            nc.sync.dma_start(out=dbg_kbT, in_=kbT)
            nc.sync.dma_start(out=dbg_vn, in_=vn)
            nc.sync.dma_start(out=dbg_aoutT, in_=aoutT.bitcast(F32))
            nc.sync.dma_start(out=dbg_xT, in_=xT)

    nc.compile()
    return nc


_CACHE = {}


def _get_nc(NSEQ):
    if NSEQ not in _CACHE:
        _CACHE[NSEQ] = build_nc(NSEQ)
    return _CACHE[NSEQ]


def kernel(x, Wq, Wk, Wv, Wo, bo):
    """Full-input entry point: shard over 8 cores, run, gather."""
    x, Wq, Wk, Wv, Wo, bo = (np.asarray(a, np.float32) for a in (x, Wq, Wk, Wv, Wo, bo))
    B, N, C = x.shape
    nc = _get_nc(N)
    in_maps = []
    for c in range(8):
        b, g = c // 4, c % 4
        in_maps.append({
            "x": np.ascontiguousarray(x[b]).astype(BF16_NP),
            "wqkv": np.ascontiguousarray(np.concatenate(
                [Wq[:, g * 256:(g + 1) * 256],
                 Wk[:, g * D:(g + 1) * D], Wv[:, g * D:(g + 1) * D]],
                axis=1)).astype(BF16_NP),
            "wo": np.ascontiguousarray(
                Wo[g * 256:(g + 1) * 256, :]).astype(BF16_NP),
        })
    res = bass_utils.run_bass_kernel_spmd(nc, in_maps, core_ids=list(range(8)))
    outs = [res.results[c]["out"] for c in range(8)]
    full = np.empty((B, N, C), np.float32)
    for b in range(B):
        acc = outs[4 * b].astype(np.float32)
        for g in range(1, 4):
            acc = acc + outs[4 * b + g]
        full[b] = acc.T + bo[None, :]
    return full
